# revision 1
# baseline (speedup 1.0000x reference)
"""Trainium2 Bass kernel for nn_Conv2d_61881888800824.

StyleGAN2-style synthesis layer:
    renorm(weight) -> upsample x2 (zero-insert) -> 4x4 FIR -> 3x3 conv
    -> + noise*strength -> + bias -> leaky_relu(0.2) * sqrt(2)

Math: the upsample+FIR+conv chain is folded (host-side) into a polyphase
decomposition — 4 independent 3x3 convolutions of the ORIGINAL 64x64 input
(one per output-pixel parity phase (alpha,beta)), each expressed as
channel-contraction matmuls on the TensorEngine in float32r (tf32-like)
precision with fp32 PSUM accumulation. The sqrt(2) lrelu gain is folded
into weights/bias/noise (lrelu is positively homogeneous).

Sharding: data-parallel over batch — 1 image per NeuronCore, 8 cores.

Self-contained: inputs are the full tensors from setup_inputs(); output is
the full [8, 256, 128, 128] fp32 array.
"""
from contextlib import ExitStack

import numpy as np

import bass_rust
import concourse.bass as bass
import concourse.mybir as mybir
import concourse.tile as tile
from concourse.bass_utils import run_bass_kernel_spmd

F32R = mybir.dt.float32r
F32 = mybir.dt.float32


# ---------------------------------------------------------------------------
# Wait legalization: this walrus build accepts at most ONE embedded sync wait
# per instruction. Tile can emit more (incl. same-engine self-waits that are
# provably satisfied by the engine's serial program order). Drop the provable
# ones; move the rest onto standalone EventSemaphore instructions inserted
# just before the over-limit instruction on the same engine.
# ---------------------------------------------------------------------------

def _is_async_update(inst) -> bool:
    n = type(inst).__name__
    return 'DMA' in n or 'Swdge' in n or 'Collective' in n or 'Dma' in n


def legalize_waits(nc, evsem_limit: int = 1) -> int:
    n_fixed = 0
    for fn in nc.m.functions:
        for bb in fn.blocks:
            insts = bb.instructions
            cum: dict[tuple, int] = {}
            out = []
            changed = False
            for inst in insts:
                si = inst.sync_info
                waits = list(si.on_wait) if si is not None and si.on_wait else []
                updates = list(si.on_update) if si is not None and si.on_update else []
                eng = inst.engine
                limit = 1
                if len(waits) > limit:
                    kept = []
                    for w in waits:
                        if (w.sync_type == 'semaphore'
                                and w.wait_mode == 'sem-ge-imm'
                                and w.wait_reg is None
                                and cum.get((eng, w.id), 0) >= w.wait_value):
                            continue
                        kept.append(w)
                    waits = kept
                if len(waits) > limit:
                    excess = waits[:-limit]
                    waits = waits[-limit:]
                    while excess:
                        take, excess = excess[:evsem_limit], excess[evsem_limit:]
                        ev = mybir.InstEventSemaphore(
                            name=nc.get_next_instruction_name(), ins=[], outs=[])
                        ev.engine = eng
                        ev.sync_info = bass_rust.SyncInfo(on_wait=take, on_update=[])
                        out.append(ev)
                    inst.sync_info = bass_rust.SyncInfo(on_wait=waits,
                                                        on_update=updates)
                    changed = True
                    n_fixed += 1
                elif si is not None and len(list(si.on_wait or [])) != len(waits):
                    inst.sync_info = bass_rust.SyncInfo(on_wait=waits,
                                                        on_update=updates)
                    changed = True
                    n_fixed += 1
                out.append(inst)
                if not _is_async_update(inst):
                    for u in updates:
                        if (u.sync_type == 'semaphore'
                                and u.update_mode == 'sem-inc'
                                and u.update_reg is None):
                            k = (eng, u.id)
                            cum[k] = cum.get(k, 0) + u.update_value
            if changed:
                bb.instructions = out
    return n_fixed


# ---------------------------------------------------------------------------
# Device kernel (per core: one batch image)
# ---------------------------------------------------------------------------

def build_conv_nc(mm_dtype=F32R):
    nc = bass.Bass("TRN2", target_bir_lowering=False, debug=False)
    xin = nc.dram_tensor("xin", [512, 64, 64], mm_dtype, kind="ExternalInput").ap()
    wq = nc.dram_tensor("wq", [2, 2, 128, 72 * 128], mm_dtype,
                        kind="ExternalInput").ap()
    noise4 = nc.dram_tensor("noise4", [2, 2, 64, 64], F32, kind="ExternalInput").ap()
    biasq = nc.dram_tensor("biasq", [128, 2], F32, kind="ExternalInput").ap()
    y = nc.dram_tensor("y", [256, 128, 128], F32, kind="ExternalOutput").ap()

    with ExitStack() as ctx:
        tc = ctx.enter_context(tile.TileContext(nc))
        xp = ctx.enter_context(tc.tile_pool(name="x", bufs=1))
        wp = ctx.enter_context(tc.tile_pool(name="w", bufs=2))
        np_ = ctx.enter_context(tc.tile_pool(name="noise", bufs=1))
        bp = ctx.enter_context(tc.tile_pool(name="bias", bufs=1))
        op = ctx.enter_context(tc.tile_pool(name="out", bufs=3))
        pp = ctx.enter_context(tc.tile_pool(name="psum", bufs=8, space="PSUM"))

        bias_sb = bp.tile([128, 2], F32)
        nc.sync.dma_start(bias_sb[:], biasq)

        # hoist the first weight slab ahead of the x loads so the SDMA
        # round-robin finishes the first-matmul critical path sooner
        wslab0 = wp.tile([128, 72, 128], mm_dtype, tag="wslab")
        wsrc0 = wq[0, 0].rearrange("ci (j co) -> ci j co", co=128)
        nc.sync.dma_start(wslab0[:, 0:36, :], wsrc0[:, 0:36, :])
        nc.sync.dma_start(wslab0[:, 36:72, :], wsrc0[:, 36:72, :])

        # x with 1-pixel zero border: [128, cg, 66, 66]; row-half DMAs give
        # finer dependency granularity for the first accumulation groups
        xq = xp.tile([128, 4, 66, 66], mm_dtype)
        for cg in range(4):
            for sl in (xq[:, cg, 0, :], xq[:, cg, 65, :],
                       xq[:, cg, :, 0], xq[:, cg, :, 65]):
                nc.vector.memset(sl.bitcast(F32), 0.0)
        for cg in range(4):
            nc.sync.dma_start(xq[:, cg, 1:33, 1:65],
                              xin[cg * 128:(cg + 1) * 128, 0:32])
            nc.sync.dma_start(xq[:, cg, 33:65, 1:65],
                              xin[cg * 128:(cg + 1) * 128, 32:64])

        for a in range(2):          # output row parity (alpha)
            noise_sb = np_.tile([128, 2, 64, 64], F32)
            nsrc = bass.AP(
                tensor=noise4.tensor,
                offset=a * 2 * 4096,
                ap=[[0, 128], [4096, 2], [64, 64], [1, 64]],
            )
            nc.sync.dma_start(noise_sb[:], nsrc)
            for ct in range(2):     # cout tile of 128
                if a == 0 and ct == 0:
                    wslab = wslab0
                else:
                    wslab = wp.tile([128, 72, 128], mm_dtype, tag="wslab")
                    wsrc = wq[a, ct].rearrange("ci (j co) -> ci j co", co=128)
                    # per beta-half: first matmuls only wait for half
                    nc.sync.dma_start(wslab[:, 0:36, :], wsrc[:, 0:36, :])
                    nc.sync.dma_start(wslab[:, 36:72, :], wsrc[:, 36:72, :])
                for c in range(8):  # chunk of 8 output-phase rows
                    ot = op.tile([128, 8, 128], F32)
                    for b in range(2):   # output col parity (beta)
                        ps = pp.tile([128, 8, 64], F32)
                        for cg in range(4):
                            for tap in range(9):
                                ky, kx = tap // 3, tap % 3
                                j = b * 36 + cg * 9 + tap
                                nc.tensor.matmul(
                                    ps[:],
                                    wslab[:, j, :],
                                    xq[:, cg, 8 * c + ky:8 * c + ky + 8,
                                       kx:kx + 64],
                                    start=(cg == 0 and tap == 0),
                                    stop=(cg == 3 and tap == 8),
                                )
                        # psum += noise; z = psum + bias -> ot (strided by beta)
                        nc.vector.tensor_add(
                            ps[:], ps[:], noise_sb[:, b, 8 * c:8 * c + 8, :])
                        nc.scalar.activation(
                            ot[:, :, b::2], ps[:],
                            mybir.ActivationFunctionType.Identity,
                            bias=bias_sb[:, ct:ct + 1], scale=1.0)
                    # leaky relu: ot = max(0.2*ot, ot)
                    nc.vector.scalar_tensor_tensor(
                        ot[:], ot[:], 0.2, ot[:],
                        mybir.AluOpType.mult, mybir.AluOpType.max)
                    ydst = bass.AP(
                        tensor=y.tensor,
                        offset=(ct * 128) * 16384 + (16 * c + a) * 128,
                        ap=[[16384, 128], [256, 8], [1, 128]],
                    )
                    nc.sync.dma_start(ydst, ot[:])
    legalize_waits(nc)
    return nc


# ---------------------------------------------------------------------------
# Host-side preparation (weight renorm + FIR folding + phase decomposition)
# ---------------------------------------------------------------------------

def prep_inputs(x, weight, bias, noise_const, noise_strength):
    SQ2 = np.sqrt(2.0)
    w = np.asarray(weight).astype(np.float64)
    inv = 1.0 / np.sqrt((w ** 2).sum(axis=(1, 2, 3)) + 1e-8)
    w = w * inv[:, None, None, None]
    f = np.array([1., 3., 3., 1.])
    f = np.outer(f, f)
    f = f / f.sum() * 4.0                       # FIR * up^2 gain
    wf = w[:, :, ::-1, ::-1]                    # flipped (cross-corr of flip)
    g = np.zeros((w.shape[0], w.shape[1], 6, 6))
    for m in range(3):
        for n in range(3):
            g[:, :, m:m + 4, n:n + 4] += wf[:, :, m, n, None, None] * f
    # wq[a, ct, ci, b*36 + cg*9 + tap, co]
    wq = np.empty((2, 2, 128, 72, 128), dtype=np.float32)
    for a in range(2):
        for b in range(2):
            h = g[:, :, (1 - a)::2, (1 - b)::2] * SQ2    # [Cout, Cin, 3, 3]
            h5 = h.reshape(2, 128, 4, 128, 9)            # [ct, co, cg, ci, tap]
            h5 = h5.transpose(0, 3, 2, 4, 1)             # [ct, ci, cg, tap, co]
            wq[a, :, :, b * 36:(b + 1) * 36, :] = h5.reshape(2, 128, 36, 128)
    wq = np.ascontiguousarray(wq.reshape(2, 2, 128, 72 * 128), dtype=np.float32)

    noise2 = np.asarray(noise_const).astype(np.float64) * float(noise_strength) * SQ2
    noise4 = np.empty((2, 2, 64, 64), dtype=np.float32)
    for a in range(2):
        for b in range(2):
            noise4[a, b] = noise2[a::2, b::2]

    biasq = np.empty((128, 2), dtype=np.float32)
    bias2 = np.asarray(bias).astype(np.float64) * SQ2
    biasq[:, 0] = bias2[:128]
    biasq[:, 1] = bias2[128:]

    x = np.asarray(x)
    return [{
        "xin": np.ascontiguousarray(x[bi], dtype=np.float32),
        "wq": wq,
        "noise4": noise4,
        "biasq": biasq,
    } for bi in range(x.shape[0])]


_NC_CACHE = None


def kernel(x, weight, bias, noise_const, noise_strength):
    global _NC_CACHE
    if _NC_CACHE is None:
        _NC_CACHE = build_conv_nc()
    in_maps = prep_inputs(x, weight, bias, noise_const, noise_strength)
    res = run_bass_kernel_spmd(_NC_CACHE, in_maps, core_ids=list(range(8)))
    return np.ascontiguousarray(
        np.stack([r["y"] for r in res.results]), dtype=np.float32)



# revision 2
# speedup vs baseline: 1.1348x; 1.1348x over previous
"""Trainium2 Bass kernel for nn_Conv2d_61881888800824 (v2: box-cascade FIR).

StyleGAN2 synthesis layer: renorm(w) -> up2 (zero-insert) -> 4x4 FIR -> 3x3
conv -> +noise -> +bias -> lrelu(0.2)*sqrt(2).

v2 factorization: by conv associativity, y = f2 (*) (wf (*) up2(x)) with
f2 = outer([1,3,3,1])/16 separable AND [1,3,3,1] = [1,1]^(*3): the channel
contraction only needs wf's original 9 taps on the COARSE 64x64 grid (4x
fewer MACs than folding the FIR into the conv), and the FIR becomes six
box passes (pure adds) on cheap engines:

  D phases (rho,sigma in {e,o}^2), coarse grid, xp[a]=x[a-1] zero-padded:
    ee: wf[1,1]@xp[k,l]
    eo: wf[1,0]@xp[k,l]   + wf[1,2]@xp[k,l+1]
    oe: wf[0,1]@xp[k,l]   + wf[2,1]@xp[k+1,l]
    oo: wf[0,0]@xp[k,l] + wf[0,2]@xp[k,l+1] + wf[2,0]@xp[k+1,l] + wf[2,2]@xp[k+1,l+1]
  then per dim: B1e[k]=De[k]+Do[k]; B1o[k]=Do[k]+De[k+1];
                B2e[k]=B1e+B1o; B2o[k]=B1o[k]+B1e[k+1];
                ye[n]=B2e+B2o;  yo[n]=B2o[n]+B2e[n+1].

Engine split per core: PE = 720 fp16 matmuls (N=462); ACT = PSUM->SBUF fp16
copies + final Lrelu(+bias, col-interleave, fp16); DVE = 6 box passes (fp16
2x mode); Pool(GPSIMD) = noise add. 1/16 FIR gain and sqrt(2) folded into
weights/bias/noise host-side.

Sharding: data-parallel over batch - 1 image per NeuronCore, 8 cores.
"""
from contextlib import ExitStack

import numpy as np

import bass_rust
import concourse.bass as bass
import concourse.mybir as mybir
import concourse.tile as tile
from concourse.bass_utils import run_bass_kernel_spmd

F16 = mybir.dt.float16
F32 = mybir.dt.float32


# ---------------------------------------------------------------------------
# Wait legalization: this walrus build accepts at most ONE embedded sync wait
# per instruction. Tile can emit more (incl. same-engine self-waits that are
# provably satisfied by the engine's serial program order). Drop the provable
# ones; move the rest onto standalone EventSemaphore instructions inserted
# just before the over-limit instruction on the same engine.
# ---------------------------------------------------------------------------

def _is_async_update(inst) -> bool:
    n = type(inst).__name__
    return 'DMA' in n or 'Swdge' in n or 'Collective' in n or 'Dma' in n


def legalize_waits(nc, evsem_limit: int = 1) -> int:
    n_fixed = 0
    for fn in nc.m.functions:
        for bb in fn.blocks:
            insts = bb.instructions
            cum: dict[tuple, int] = {}
            out = []
            changed = False
            for inst in insts:
                si = inst.sync_info
                waits = list(si.on_wait) if si is not None and si.on_wait else []
                updates = list(si.on_update) if si is not None and si.on_update else []
                eng = inst.engine
                limit = 1
                if len(waits) > limit:
                    kept = []
                    for w in waits:
                        if (w.sync_type == 'semaphore'
                                and w.wait_mode == 'sem-ge-imm'
                                and w.wait_reg is None
                                and cum.get((eng, w.id), 0) >= w.wait_value):
                            continue
                        kept.append(w)
                    waits = kept
                if len(waits) > limit:
                    excess = waits[:-limit]
                    waits = waits[-limit:]
                    while excess:
                        take, excess = excess[:evsem_limit], excess[evsem_limit:]
                        ev = mybir.InstEventSemaphore(
                            name=nc.get_next_instruction_name(), ins=[], outs=[])
                        ev.engine = eng
                        ev.sync_info = bass_rust.SyncInfo(on_wait=take, on_update=[])
                        out.append(ev)
                    inst.sync_info = bass_rust.SyncInfo(on_wait=waits,
                                                        on_update=updates)
                    changed = True
                    n_fixed += 1
                elif si is not None and len(list(si.on_wait or [])) != len(waits):
                    inst.sync_info = bass_rust.SyncInfo(on_wait=waits,
                                                        on_update=updates)
                    changed = True
                    n_fixed += 1
                out.append(inst)
                if not _is_async_update(inst):
                    for u in updates:
                        if (u.sync_type == 'semaphore'
                                and u.update_mode == 'sem-inc'
                                and u.update_reg is None):
                            k = (eng, u.id)
                            cum[k] = cum.get(k, 0) + u.update_value
            if changed:
                bb.instructions = out
    return n_fixed


# ---------------------------------------------------------------------------
# Device kernel (per core: one batch image)
# ---------------------------------------------------------------------------

# phase (rho, sigma) -> [(u, v, dr, dc)] tap list
TAPS = {
    (0, 0): [(1, 1, 0, 0)],
    (0, 1): [(1, 0, 0, 0), (1, 2, 0, 1)],
    (1, 0): [(0, 1, 0, 0), (2, 1, 1, 0)],
    (1, 1): [(0, 0, 0, 0), (0, 2, 0, 1), (2, 0, 1, 0), (2, 2, 1, 1)],
}
CHUNKS = [(c * 7, min(7, 66 - c * 7)) for c in range(10)]   # (k0, nrows)
PRELU = mybir.ActivationFunctionType.Prelu
COPY = mybir.ActivationFunctionType.Copy


def build_conv_nc(legalize=True, epilogue='prelu'):
    nc = bass.Bass("TRN2", target_bir_lowering=False, debug=False)
    xin = nc.dram_tensor("xin", [4, 128, 67, 67], F16, kind="ExternalInput").ap()
    wq = nc.dram_tensor("wq", [4, 128, 9, 256], F16, kind="ExternalInput").ap()
    # nq[ct, p, r, s*64+m] = noise[r, 2m+s]*strength*sqrt2 + bias[ct*128+p]*sqrt2
    nq = nc.dram_tensor("nq", [2, 128, 128, 128], F16, kind="ExternalInput").ap()
    y = nc.dram_tensor("y", [2, 128, 128, 128], F16, kind="ExternalOutput").ap()

    with ExitStack() as ctx:
        tc = ctx.enter_context(tile.TileContext(nc))
        xpool = ctx.enter_context(tc.tile_pool(name="x", bufs=1))
        wpool = ctx.enter_context(tc.tile_pool(name="w", bufs=1))
        npool = ctx.enter_context(tc.tile_pool(name="noise", bufs=1))
        work = ctx.enter_context(tc.tile_pool(name="work", bufs=3))
        ypool = ctx.enter_context(tc.tile_pool(name="yb", bufs=2))
        pp = ctx.enter_context(tc.tile_pool(name="psum", bufs=8, space="PSUM"))

        wsb = wpool.tile([128, 9, 4, 256], F16)
        for cg in range(4):
            nc.sync.dma_start(wsb[:, :, cg, :], wq[cg])

        xq = xpool.tile([128, 4, 67, 67], F16)
        for cg in range(4):
            nc.sync.dma_start(xq[:, cg, 0:16, :], xin[cg, :, 0:16, :])
            nc.sync.dma_start(xq[:, cg, 16:34, :], xin[cg, :, 16:34, :])
            nc.sync.dma_start(xq[:, cg, 34:50, :], xin[cg, :, 34:50, :])
            nc.sync.dma_start(xq[:, cg, 50:67, :], xin[cg, :, 50:67, :])

        for ct in range(2):
            # per-ct noise+bias tile (reloaded: bias varies with ct)
            nsb = npool.tile([128, 128, 128], F16, tag="noise")
            nc.sync.dma_start(nsb[:], nq[ct])
            # --- conv matmuls into per-phase PSUM chunks; ACT drains to D ---
            # D flat layout: [128, rho, k(66), sigma*66 + l (132)]
            D = work.tile([128, 2, 66, 132], F16, tag="work")
            for (k0, nr) in CHUNKS:
                for (rho, sg), taps in TAPS.items():
                    ps = pp.tile([128, 7, 66], F32, tag="psum")
                    nmm = len(taps) * 4
                    i = 0
                    for (u, v, dr, dc) in taps:
                        for cg in range(4):
                            nc.tensor.matmul(
                                ps[:, 0:nr, :],
                                wsb[:, u * 3 + v, cg, ct * 128:(ct + 1) * 128],
                                xq[:, cg, k0 + dr:k0 + dr + nr, dc:dc + 66],
                                start=(i == 0), stop=(i == nmm - 1))
                            i += 1
                    nc.scalar.activation(
                        D[:, rho, k0:k0 + nr, sg * 66:sg * 66 + 66],
                        ps[:, 0:nr, :], COPY)

            # --- row box cascade (DVE, fp16 2x), band-split so each stage
            # pipelines behind the previous as a wavefront (region deps) ---
            B1 = work.tile([128, 2, 66, 132], F16, tag="work")
            for (a, b) in ((0, 33), (33, 65)):
                nc.vector.tensor_add(B1[:, 0, a:b, :], D[:, 0, a:b, :],
                                     D[:, 1, a:b, :])
                nc.vector.tensor_add(B1[:, 1, a:b, :], D[:, 1, a:b, :],
                                     D[:, 0, a + 1:b + 1, :])
            B2 = work.tile([128, 2, 66, 132], F16, tag="work")
            for (a, b, b1) in ((0, 33, 33), (33, 65, 64)):
                nc.vector.tensor_add(B2[:, 0, a:b, :], B1[:, 0, a:b, :],
                                     B1[:, 1, a:b, :])
                nc.vector.tensor_add(B2[:, 1, a:b1, :], B1[:, 1, a:b1, :],
                                     B1[:, 0, a + 1:b1 + 1, :])
            R = work.tile([128, 128, 132], F16, tag="work")
            for (a, b) in ((0, 32), (32, 64)):
                nc.vector.tensor_add(R[:, 2 * a:2 * b:2, :], B2[:, 0, a:b, :],
                                     B2[:, 1, a:b, :])
                nc.vector.tensor_add(R[:, 2 * a + 1:2 * b:2, :],
                                     B2[:, 1, a:b, :], B2[:, 0, a + 1:b + 1, :])

            # --- col box cascade (flat col blocks: e = [0:66], o = [66:132]) ---
            C1 = work.tile([128, 128, 132], F16, tag="work")
            C2 = work.tile([128, 128, 132], F16, tag="work")
            C3 = work.tile([128, 128, 132], F16, tag="work")
            for (a, b) in ((0, 64), (64, 128)):
                nc.vector.tensor_add(C1[:, a:b, 0:65], R[:, a:b, 0:65],
                                     R[:, a:b, 66:131])
                nc.vector.tensor_add(C1[:, a:b, 66:131], R[:, a:b, 66:131],
                                     R[:, a:b, 1:66])
            for (a, b) in ((0, 64), (64, 128)):
                nc.vector.tensor_add(C2[:, a:b, 0:65], C1[:, a:b, 0:65],
                                     C1[:, a:b, 66:131])
                nc.vector.tensor_add(C2[:, a:b, 66:130], C1[:, a:b, 66:130],
                                     C1[:, a:b, 1:65])
            for (a, b) in ((0, 64), (64, 128)):
                nc.vector.tensor_add(C3[:, a:b, 0:64], C2[:, a:b, 0:64],
                                     C2[:, a:b, 66:130])
                nc.vector.tensor_add(C3[:, a:b, 66:130], C2[:, a:b, 66:130],
                                     C2[:, a:b, 1:65])

            # --- noise+bias add (Pool, in place on C3); lrelu + col
            # interleave + fp16 narrowing fused on ACT; band DMA out ---
            for b in range(4):
                r0 = b * 32
                Y = ypool.tile([128, 32, 128], F16, tag="yb")
                for sg in range(2):
                    cs = slice(sg * 66, sg * 66 + 64)
                    nc.gpsimd.tensor_add(C3[:, r0:r0 + 32, cs],
                                         C3[:, r0:r0 + 32, cs],
                                         nsb[:, r0:r0 + 32, sg * 64:sg * 64 + 64])
                    if epilogue == 'prelu':
                        nc.scalar.activation(
                            Y[:, :, sg::2], C3[:, r0:r0 + 32, cs],
                            PRELU, bias=0.0, scale=1.0, alpha=0.2)
                    else:
                        nc.vector.scalar_tensor_tensor(
                            C3[:, r0:r0 + 32, cs], C3[:, r0:r0 + 32, cs], 0.2,
                            C3[:, r0:r0 + 32, cs],
                            mybir.AluOpType.mult, mybir.AluOpType.max)
                        nc.scalar.activation(
                            Y[:, :, sg::2], C3[:, r0:r0 + 32, cs], COPY)
                nc.sync.dma_start(y[ct, :, r0:r0 + 32, :], Y[:])
    if legalize:
        legalize_waits(nc)
    return nc


# ---------------------------------------------------------------------------
# Host-side preparation
# ---------------------------------------------------------------------------

def prep_inputs(x, weight, bias, noise_const, noise_strength):
    SQ2 = np.sqrt(2.0)
    w = np.asarray(weight, np.float64)
    inv = 1.0 / np.sqrt((w ** 2).sum(axis=(1, 2, 3)) + 1e-8)
    w = w * inv[:, None, None, None]
    wf = w[:, :, ::-1, ::-1] * (SQ2 / 16.0)       # fold FIR norm + lrelu gain
    # wq[cg, ci, u*3+v, co]
    wq = np.ascontiguousarray(
        wf.transpose(1, 2, 3, 0).reshape(4, 128, 9, 256), dtype=np.float16)

    n2 = np.asarray(noise_const, np.float64) * float(noise_strength) * SQ2
    # noise part: [r, s*64 + m] = n2[r, 2m+s]; plus per-partition bias per ct
    nflat = n2.reshape(128, 64, 2).transpose(0, 2, 1).reshape(128, 128)
    b2 = (np.asarray(bias, np.float64) * SQ2).reshape(2, 128)
    nq = np.ascontiguousarray(
        nflat[None, None, :, :] + b2[:, :, None, None], dtype=np.float16)

    x = np.asarray(x, np.float32)
    maps = []
    for bi in range(x.shape[0]):
        xp = np.zeros((512, 67, 67), np.float16)
        xp[:, 1:65, 1:65] = x[bi]
        maps.append({
            "xin": xp.reshape(4, 128, 67, 67),
            "wq": wq,
            "nq": nq,
        })
    return maps


_NC_CACHE = None


def kernel(x, weight, bias, noise_const, noise_strength):
    global _NC_CACHE
    if _NC_CACHE is None:
        _NC_CACHE = build_conv_nc()
    in_maps = prep_inputs(x, weight, bias, noise_const, noise_strength)
    res = run_bass_kernel_spmd(_NC_CACHE, in_maps, core_ids=list(range(8)))
    return np.ascontiguousarray(
        np.stack([r["y"].reshape(256, 128, 128) for r in res.results]),
        dtype=np.float32)


# revision 3
# speedup vs baseline: 1.1579x; 1.0204x over previous
"""Trainium2 Bass kernel for nn_Conv2d_61881888800824 (v2: box-cascade FIR).

StyleGAN2 synthesis layer: renorm(w) -> up2 (zero-insert) -> 4x4 FIR -> 3x3
conv -> +noise -> +bias -> lrelu(0.2)*sqrt(2).

v2 factorization: by conv associativity, y = f2 (*) (wf (*) up2(x)) with
f2 = outer([1,3,3,1])/16 separable AND [1,3,3,1] = [1,1]^(*3): the channel
contraction only needs wf's original 9 taps on the COARSE 64x64 grid (4x
fewer MACs than folding the FIR into the conv), and the FIR becomes six
box passes (pure adds) on cheap engines:

  D phases (rho,sigma in {e,o}^2), coarse grid, xp[a]=x[a-1] zero-padded:
    ee: wf[1,1]@xp[k,l]
    eo: wf[1,0]@xp[k,l]   + wf[1,2]@xp[k,l+1]
    oe: wf[0,1]@xp[k,l]   + wf[2,1]@xp[k+1,l]
    oo: wf[0,0]@xp[k,l] + wf[0,2]@xp[k,l+1] + wf[2,0]@xp[k+1,l] + wf[2,2]@xp[k+1,l+1]
  then per dim: B1e[k]=De[k]+Do[k]; B1o[k]=Do[k]+De[k+1];
                B2e[k]=B1e+B1o; B2o[k]=B1o[k]+B1e[k+1];
                ye[n]=B2e+B2o;  yo[n]=B2o[n]+B2e[n+1].

Engine split per core: PE = 720 fp16 matmuls (N=462); ACT = PSUM->SBUF fp16
copies + final Lrelu(+bias, col-interleave, fp16); DVE = 6 box passes (fp16
2x mode); Pool(GPSIMD) = noise add. 1/16 FIR gain and sqrt(2) folded into
weights/bias/noise host-side.

Sharding: data-parallel over batch - 1 image per NeuronCore, 8 cores.
"""
from contextlib import ExitStack

import numpy as np

import bass_rust
import concourse.bass as bass
import concourse.mybir as mybir
import concourse.tile as tile
from concourse.bass_utils import run_bass_kernel_spmd

F16 = mybir.dt.float16
F32 = mybir.dt.float32


# ---------------------------------------------------------------------------
# Wait legalization: this walrus build accepts at most ONE embedded sync wait
# per instruction. Tile can emit more (incl. same-engine self-waits that are
# provably satisfied by the engine's serial program order). Drop the provable
# ones; move the rest onto standalone EventSemaphore instructions inserted
# just before the over-limit instruction on the same engine.
# ---------------------------------------------------------------------------

def _is_async_update(inst) -> bool:
    n = type(inst).__name__
    return 'DMA' in n or 'Swdge' in n or 'Collective' in n or 'Dma' in n


def legalize_waits(nc, evsem_limit: int = 1) -> int:
    n_fixed = 0
    for fn in nc.m.functions:
        for bb in fn.blocks:
            insts = bb.instructions
            cum: dict[tuple, int] = {}
            out = []
            changed = False
            for inst in insts:
                si = inst.sync_info
                waits = list(si.on_wait) if si is not None and si.on_wait else []
                updates = list(si.on_update) if si is not None and si.on_update else []
                eng = inst.engine
                limit = 1
                if len(waits) > limit:
                    kept = []
                    for w in waits:
                        if (w.sync_type == 'semaphore'
                                and w.wait_mode == 'sem-ge-imm'
                                and w.wait_reg is None
                                and cum.get((eng, w.id), 0) >= w.wait_value):
                            continue
                        kept.append(w)
                    waits = kept
                if len(waits) > limit:
                    excess = waits[:-limit]
                    waits = waits[-limit:]
                    while excess:
                        take, excess = excess[:evsem_limit], excess[evsem_limit:]
                        ev = mybir.InstEventSemaphore(
                            name=nc.get_next_instruction_name(), ins=[], outs=[])
                        ev.engine = eng
                        ev.sync_info = bass_rust.SyncInfo(on_wait=take, on_update=[])
                        out.append(ev)
                    inst.sync_info = bass_rust.SyncInfo(on_wait=waits,
                                                        on_update=updates)
                    changed = True
                    n_fixed += 1
                elif si is not None and len(list(si.on_wait or [])) != len(waits):
                    inst.sync_info = bass_rust.SyncInfo(on_wait=waits,
                                                        on_update=updates)
                    changed = True
                    n_fixed += 1
                out.append(inst)
                if not _is_async_update(inst):
                    for u in updates:
                        if (u.sync_type == 'semaphore'
                                and u.update_mode == 'sem-inc'
                                and u.update_reg is None):
                            k = (eng, u.id)
                            cum[k] = cum.get(k, 0) + u.update_value
            if changed:
                bb.instructions = out
    return n_fixed


# ---------------------------------------------------------------------------
# Device kernel (per core: one batch image)
# ---------------------------------------------------------------------------

# phase (rho, sigma) -> [(u, v, dr, dc)] tap list
TAPS = {
    (0, 0): [(1, 1, 0, 0)],
    (0, 1): [(1, 0, 0, 0), (1, 2, 0, 1)],
    (1, 0): [(0, 1, 0, 0), (2, 1, 1, 0)],
    (1, 1): [(0, 0, 0, 0), (0, 2, 0, 1), (2, 0, 1, 0), (2, 2, 1, 1)],
}
CHUNKS = [(c * 7, min(7, 66 - c * 7)) for c in range(10)]   # (k0, nrows)
PRELU = mybir.ActivationFunctionType.Prelu
COPY = mybir.ActivationFunctionType.Copy


def build_conv_nc(legalize=True, epilogue='prelu'):
    nc = bass.Bass("TRN2", target_bir_lowering=False, debug=False)
    xin = nc.dram_tensor("xin", [4, 128, 67, 67], F16, kind="ExternalInput").ap()
    wq = nc.dram_tensor("wq", [4, 128, 9, 256], F16, kind="ExternalInput").ap()
    # nq[ct, p, r, s*64+m] = noise[r, 2m+s]*strength*sqrt2 + bias[ct*128+p]*sqrt2
    nq = nc.dram_tensor("nq", [2, 128, 128, 128], F16, kind="ExternalInput").ap()
    y = nc.dram_tensor("y", [2, 128, 128, 128], F16, kind="ExternalOutput").ap()

    with ExitStack() as ctx:
        tc = ctx.enter_context(tile.TileContext(nc))
        xpool = ctx.enter_context(tc.tile_pool(name="x", bufs=1))
        wpool = ctx.enter_context(tc.tile_pool(name="w", bufs=1))
        work = ctx.enter_context(tc.tile_pool(name="work", bufs=6))
        nypool = ctx.enter_context(tc.tile_pool(name="ny", bufs=3))
        pp = ctx.enter_context(tc.tile_pool(name="psum", bufs=8, space="PSUM"))

        wsb = wpool.tile([128, 9, 4, 256], F16)
        for cg in range(4):
            nc.sync.dma_start(wsb[:, :, cg, :], wq[cg])

        xq = xpool.tile([128, 4, 67, 67], F16)
        for cg in range(4):
            nc.sync.dma_start(xq[:, cg, 0:16, :], xin[cg, :, 0:16, :])
            nc.sync.dma_start(xq[:, cg, 16:34, :], xin[cg, :, 16:34, :])
            nc.sync.dma_start(xq[:, cg, 34:50, :], xin[cg, :, 34:50, :])
            nc.sync.dma_start(xq[:, cg, 50:67, :], xin[cg, :, 50:67, :])

        # 4 units = (ct, half). Half A: D rows 0..35 (local=global), y rows
        # 0..63. Half B: D rows 32..65 (local=global-32), y rows 64..127.
        # Rows 32..35 of D are copied into BOTH halves (from half A's k0=30
        # chunk) so the FIR never reads across unit tiles.
        def emit_unit_mm(ct, half, D):
            """Matmuls + PSUM->SBUF fp16 copies into the given D tile."""
            if half == 0:
                chunks = [(k0, 6) for k0 in range(0, 36, 6)]
                dbase = 0
            else:
                chunks = [(k0, 6) for k0 in range(36, 66, 6)]
                dbase = 32
            for (k0, nr) in chunks:
                for (rho, sg), taps in TAPS.items():
                    ps = pp.tile([128, 6, 66], F32, tag="psum")
                    nmm = len(taps) * 4
                    i = 0
                    for (u, v, dr, dc) in taps:
                        for cg in range(4):
                            nc.tensor.matmul(
                                ps[:, 0:nr, :],
                                wsb[:, u * 3 + v, cg, ct * 128:(ct + 1) * 128],
                                xq[:, cg, k0 + dr:k0 + dr + nr, dc:dc + 66],
                                start=(i == 0), stop=(i == nmm - 1))
                            i += 1
                    lk = k0 - dbase
                    nc.scalar.activation(
                        D[:, rho, lk:lk + nr, sg * 66:sg * 66 + 66],
                        ps[:, 0:nr, :], COPY)
                    if half == 0 and k0 == 30:
                        # dup rows 32..35 into the SAME ct's half-B D tile
                        nc.scalar.activation(
                            dup_targets[ct][:, rho, 0:4, sg * 66:sg * 66 + 66],
                            ps[:, 2:6, :], COPY)

        def emit_unit_fir(D, half):
            """Box cascade for one unit. Returns C3 [128, 64, 132]."""
            n1 = 34 if half == 0 else 33       # B1 rows
            B1 = work.tile([128, 2, 34, 132], F16, tag="work")
            nc.vector.tensor_add(B1[:, 0, 0:n1, :], D[:, 0, 0:n1, :],
                                 D[:, 1, 0:n1, :])
            nc.vector.tensor_add(B1[:, 1, 0:n1, :], D[:, 1, 0:n1, :],
                                 D[:, 0, 1:n1 + 1, :])
            B2 = work.tile([128, 2, 33, 132], F16, tag="work")
            nc.vector.tensor_add(B2[:, 0, 0:33, :], B1[:, 0, 0:33, :],
                                 B1[:, 1, 0:33, :])
            n2 = 33 if half == 0 else 32       # B2 rho1 rows
            nc.vector.tensor_add(B2[:, 1, 0:n2, :], B1[:, 1, 0:n2, :],
                                 B1[:, 0, 1:n2 + 1, :])
            R = work.tile([128, 64, 132], F16, tag="work")
            nc.vector.tensor_add(R[:, 0:64:2, :], B2[:, 0, 0:32, :],
                                 B2[:, 1, 0:32, :])
            nc.vector.tensor_add(R[:, 1:64:2, :], B2[:, 1, 0:32, :],
                                 B2[:, 0, 1:33, :])
            C1 = work.tile([128, 64, 132], F16, tag="work")
            nc.vector.tensor_add(C1[:, :, 0:65], R[:, :, 0:65], R[:, :, 66:131])
            nc.vector.tensor_add(C1[:, :, 66:131], R[:, :, 66:131], R[:, :, 1:66])
            C2 = work.tile([128, 64, 132], F16, tag="work")
            nc.vector.tensor_add(C2[:, :, 0:65], C1[:, :, 0:65], C1[:, :, 66:131])
            nc.vector.tensor_add(C2[:, :, 66:130], C1[:, :, 66:130],
                                 C1[:, :, 1:65])
            C3 = work.tile([128, 64, 132], F16, tag="work")
            nc.vector.tensor_add(C3[:, :, 0:64], C2[:, :, 0:64], C2[:, :, 66:130])
            nc.vector.tensor_add(C3[:, :, 66:130], C2[:, :, 66:130],
                                 C2[:, :, 1:65])
            return C3

        def emit_unit_epilogue(ct, half, C3):
            """noise+bias on Pool (in place), prelu+interleave+fp16 on ACT
            writing over the noise band tile, DMA out. 2 sub-bands."""
            for b in range(2):
                r0 = b * 32
                g0 = half * 64 + r0
                nyb = nypool.tile([128, 32, 128], F16, tag="ny")
                nc.sync.dma_start(nyb[:], nq[ct, :, g0:g0 + 32, :])
                for sg in range(2):
                    cs = slice(sg * 66, sg * 66 + 64)
                    nc.gpsimd.tensor_add(C3[:, r0:r0 + 32, cs],
                                         C3[:, r0:r0 + 32, cs],
                                         nyb[:, :, sg * 64:sg * 64 + 64])
                for sg in range(2):
                    cs = slice(sg * 66, sg * 66 + 64)
                    if epilogue == 'prelu':
                        nc.scalar.activation(
                            nyb[:, :, sg::2], C3[:, r0:r0 + 32, cs],
                            PRELU, bias=0.0, scale=1.0, alpha=0.2)
                    else:
                        nc.vector.scalar_tensor_tensor(
                            C3[:, r0:r0 + 32, cs], C3[:, r0:r0 + 32, cs], 0.2,
                            C3[:, r0:r0 + 32, cs],
                            mybir.AluOpType.mult, mybir.AluOpType.max)
                        nc.scalar.activation(
                            nyb[:, :, sg::2], C3[:, r0:r0 + 32, cs], COPY)
                nc.sync.dma_start(y[ct, :, g0:g0 + 32, :], nyb[:])

        # pre-allocate half-B D tiles (receive dup rows during half-A mms)
        dup_targets = {}
        units = [(0, 0), (0, 1), (1, 0), (1, 1)]
        pending = []     # (ct, half, C3) awaiting epilogue
        for (ct, half) in units:
            if half == 0:
                D = work.tile([128, 2, 36, 132], F16, tag="work")
                dup_targets[ct] = work.tile([128, 2, 34, 132], F16, tag="work", name=f"dupD{ct}")
            else:
                D = dup_targets[ct]
            emit_unit_mm(ct, half, D)
            C3 = emit_unit_fir(D, half)
            pending.append((ct, half, C3))
            if len(pending) > 1:
                pct, ph, pC3 = pending.pop(0)
                emit_unit_epilogue(pct, ph, pC3)
        for (pct, ph, pC3) in pending:
            emit_unit_epilogue(pct, ph, pC3)
    if legalize:
        legalize_waits(nc)
    return nc


# ---------------------------------------------------------------------------
# Host-side preparation
# ---------------------------------------------------------------------------

def prep_inputs(x, weight, bias, noise_const, noise_strength):
    SQ2 = np.sqrt(2.0)
    w = np.asarray(weight, np.float64)
    inv = 1.0 / np.sqrt((w ** 2).sum(axis=(1, 2, 3)) + 1e-8)
    w = w * inv[:, None, None, None]
    wf = w[:, :, ::-1, ::-1] * (SQ2 / 16.0)       # fold FIR norm + lrelu gain
    # wq[cg, ci, u*3+v, co]
    wq = np.ascontiguousarray(
        wf.transpose(1, 2, 3, 0).reshape(4, 128, 9, 256), dtype=np.float16)

    n2 = np.asarray(noise_const, np.float64) * float(noise_strength) * SQ2
    # noise part: [r, s*64 + m] = n2[r, 2m+s]; plus per-partition bias per ct
    nflat = n2.reshape(128, 64, 2).transpose(0, 2, 1).reshape(128, 128)
    b2 = (np.asarray(bias, np.float64) * SQ2).reshape(2, 128)
    nq = np.ascontiguousarray(
        nflat[None, None, :, :] + b2[:, :, None, None], dtype=np.float16)

    x = np.asarray(x, np.float32)
    maps = []
    for bi in range(x.shape[0]):
        xp = np.zeros((512, 67, 67), np.float16)
        xp[:, 1:65, 1:65] = x[bi]
        maps.append({
            "xin": xp.reshape(4, 128, 67, 67),
            "wq": wq,
            "nq": nq,
        })
    return maps


_NC_CACHE = None


def kernel(x, weight, bias, noise_const, noise_strength):
    global _NC_CACHE
    if _NC_CACHE is None:
        _NC_CACHE = build_conv_nc()
    in_maps = prep_inputs(x, weight, bias, noise_const, noise_strength)
    res = run_bass_kernel_spmd(_NC_CACHE, in_maps, core_ids=list(range(8)))
    return np.ascontiguousarray(
        np.stack([r["y"].reshape(256, 128, 128) for r in res.results]),
        dtype=np.float32)


# revision 4
# speedup vs baseline: 1.2221x; 1.0554x over previous
"""Trainium2 Bass kernel for nn_Conv2d_61881888800824 (v2: box-cascade FIR).

StyleGAN2 synthesis layer: renorm(w) -> up2 (zero-insert) -> 4x4 FIR -> 3x3
conv -> +noise -> +bias -> lrelu(0.2)*sqrt(2).

v2 factorization: by conv associativity, y = f2 (*) (wf (*) up2(x)) with
f2 = outer([1,3,3,1])/16 separable AND [1,3,3,1] = [1,1]^(*3): the channel
contraction only needs wf's original 9 taps on the COARSE 64x64 grid (4x
fewer MACs than folding the FIR into the conv), and the FIR becomes six
box passes (pure adds) on cheap engines:

  D phases (rho,sigma in {e,o}^2), coarse grid, xp[a]=x[a-1] zero-padded:
    ee: wf[1,1]@xp[k,l]
    eo: wf[1,0]@xp[k,l]   + wf[1,2]@xp[k,l+1]
    oe: wf[0,1]@xp[k,l]   + wf[2,1]@xp[k+1,l]
    oo: wf[0,0]@xp[k,l] + wf[0,2]@xp[k,l+1] + wf[2,0]@xp[k+1,l] + wf[2,2]@xp[k+1,l+1]
  then per dim: B1e[k]=De[k]+Do[k]; B1o[k]=Do[k]+De[k+1];
                B2e[k]=B1e+B1o; B2o[k]=B1o[k]+B1e[k+1];
                ye[n]=B2e+B2o;  yo[n]=B2o[n]+B2e[n+1].

Engine split per core: PE = 720 fp16 matmuls (N=462); ACT = PSUM->SBUF fp16
copies + final Lrelu(+bias, col-interleave, fp16); DVE = 6 box passes (fp16
2x mode); Pool(GPSIMD) = noise add. 1/16 FIR gain and sqrt(2) folded into
weights/bias/noise host-side.

Sharding: data-parallel over batch - 1 image per NeuronCore, 8 cores.
"""
from contextlib import ExitStack

import numpy as np

import bass_rust
import concourse.bass as bass
import concourse.mybir as mybir
import concourse.tile as tile
from concourse.bass_utils import run_bass_kernel_spmd

F16 = mybir.dt.float16
F32 = mybir.dt.float32


# ---------------------------------------------------------------------------
# Wait legalization: this walrus build accepts at most ONE embedded sync wait
# per instruction. Tile can emit more (incl. same-engine self-waits that are
# provably satisfied by the engine's serial program order). Drop the provable
# ones; move the rest onto standalone EventSemaphore instructions inserted
# just before the over-limit instruction on the same engine.
# ---------------------------------------------------------------------------

def _is_async_update(inst) -> bool:
    n = type(inst).__name__
    return 'DMA' in n or 'Swdge' in n or 'Collective' in n or 'Dma' in n


def legalize_waits(nc, evsem_limit: int = 1) -> int:
    n_fixed = 0
    for fn in nc.m.functions:
        for bb in fn.blocks:
            insts = bb.instructions
            cum: dict[tuple, int] = {}
            out = []
            changed = False
            for inst in insts:
                si = inst.sync_info
                waits = list(si.on_wait) if si is not None and si.on_wait else []
                updates = list(si.on_update) if si is not None and si.on_update else []
                eng = inst.engine
                limit = 1
                if len(waits) > limit:
                    kept = []
                    for w in waits:
                        if (w.sync_type == 'semaphore'
                                and w.wait_mode == 'sem-ge-imm'
                                and w.wait_reg is None
                                and cum.get((eng, w.id), 0) >= w.wait_value):
                            continue
                        kept.append(w)
                    waits = kept
                if len(waits) > limit:
                    excess = waits[:-limit]
                    waits = waits[-limit:]
                    while excess:
                        take, excess = excess[:evsem_limit], excess[evsem_limit:]
                        ev = mybir.InstEventSemaphore(
                            name=nc.get_next_instruction_name(), ins=[], outs=[])
                        ev.engine = eng
                        ev.sync_info = bass_rust.SyncInfo(on_wait=take, on_update=[])
                        out.append(ev)
                    inst.sync_info = bass_rust.SyncInfo(on_wait=waits,
                                                        on_update=updates)
                    changed = True
                    n_fixed += 1
                elif si is not None and len(list(si.on_wait or [])) != len(waits):
                    inst.sync_info = bass_rust.SyncInfo(on_wait=waits,
                                                        on_update=updates)
                    changed = True
                    n_fixed += 1
                out.append(inst)
                if not _is_async_update(inst):
                    for u in updates:
                        if (u.sync_type == 'semaphore'
                                and u.update_mode == 'sem-inc'
                                and u.update_reg is None):
                            k = (eng, u.id)
                            cum[k] = cum.get(k, 0) + u.update_value
            if changed:
                bb.instructions = out
    return n_fixed


# ---------------------------------------------------------------------------
# Device kernel (per core: one batch image)
# ---------------------------------------------------------------------------

# phase (rho, sigma) -> [(u, v, dr, dc)] tap list
TAPS = {
    (0, 0): [(1, 1, 0, 0)],
    (0, 1): [(1, 0, 0, 0), (1, 2, 0, 1)],
    (1, 0): [(0, 1, 0, 0), (2, 1, 1, 0)],
    (1, 1): [(0, 0, 0, 0), (0, 2, 0, 1), (2, 0, 1, 0), (2, 2, 1, 1)],
}
CHUNKS = [(c * 7, min(7, 66 - c * 7)) for c in range(10)]   # (k0, nrows)
PRELU = mybir.ActivationFunctionType.Prelu
COPY = mybir.ActivationFunctionType.Copy


def build_conv_nc(legalize=True, epilogue='prelu'):
    nc = bass.Bass("TRN2", target_bir_lowering=False, debug=False)
    xin = nc.dram_tensor("xin", [4, 128, 67, 67], F16, kind="ExternalInput").ap()
    wq = nc.dram_tensor("wq", [4, 128, 9, 256], F16, kind="ExternalInput").ap()
    # nq[ct, p, r, s*64+m] = noise[r, 2m+s]*strength*sqrt2 + bias[ct*128+p]*sqrt2
    nq = nc.dram_tensor("nq", [2, 128, 128, 128], F16, kind="ExternalInput").ap()
    y = nc.dram_tensor("y", [2, 128, 128, 128], F16, kind="ExternalOutput").ap()

    with ExitStack() as ctx:
        tc = ctx.enter_context(tile.TileContext(nc))
        xpool = ctx.enter_context(tc.tile_pool(name="x", bufs=1))
        wpool = ctx.enter_context(tc.tile_pool(name="w", bufs=1))
        work = ctx.enter_context(tc.tile_pool(name="work", bufs=6))
        nypool = ctx.enter_context(tc.tile_pool(name="ny", bufs=3))
        pp = ctx.enter_context(tc.tile_pool(name="psum", bufs=8, space="PSUM"))

        wsb = wpool.tile([128, 9, 4, 256], F16)
        for cg in range(4):
            nc.sync.dma_start(wsb[:, :, cg, :], wq[cg])

        xq = xpool.tile([128, 4, 67, 67], F16)
        for (r0, r1) in ((0, 16), (16, 34), (34, 50), (50, 67)):
            for cg in range(4):
                nc.sync.dma_start(xq[:, cg, r0:r1, :], xin[cg, :, r0:r1, :])

        # 4 units = (ct, half). Half A: D rows 0..35 (local=global), y rows
        # 0..63. Half B: D rows 32..65 (local=global-32), y rows 64..127.
        # Rows 32..35 of D are copied into BOTH halves (from half A's k0=30
        # chunk) so the FIR never reads across unit tiles.
        def emit_unit_mm(ct, half, D):
            """Matmuls + PSUM->SBUF fp16 copies into the given D tile."""
            if half == 0:
                chunks = [(k0, 6) for k0 in range(0, 36, 6)]
                dbase = 0
            else:
                chunks = [(k0, 6) for k0 in range(36, 66, 6)]
                dbase = 32
            for (k0, nr) in chunks:
                for (rho, sg), taps in TAPS.items():
                    ps = pp.tile([128, 6, 66], F32, tag="psum")
                    nmm = len(taps) * 4
                    i = 0
                    for (u, v, dr, dc) in taps:
                        for cg in range(4):
                            nc.tensor.matmul(
                                ps[:, 0:nr, :],
                                wsb[:, u * 3 + v, cg, ct * 128:(ct + 1) * 128],
                                xq[:, cg, k0 + dr:k0 + dr + nr, dc:dc + 66],
                                start=(i == 0), stop=(i == nmm - 1))
                            i += 1
                    lk = k0 - dbase
                    nc.scalar.activation(
                        D[:, rho, lk:lk + nr, sg * 66:sg * 66 + 66],
                        ps[:, 0:nr, :], COPY)
                    if half == 0 and k0 == 30:
                        # dup rows 32..35 into the SAME ct's half-B D tile
                        nc.scalar.activation(
                            dup_targets[ct][:, rho, 0:4, sg * 66:sg * 66 + 66],
                            ps[:, 2:6, :], COPY)

        def emit_unit_fir(D, half, nsplit=1):
            """Box cascade for one unit. Returns C3 [128, 64, 132].

            nsplit=2 sub-bands each stage op so the cascade wavefronts
            behind this unit's own matmul stream (used for the last unit,
            whose chain otherwise runs entirely after PE finishes)."""
            n1 = 34 if half == 0 else 33       # B1 rows
            n2 = 33 if half == 0 else 32       # B2 rho1 rows

            def bands(n):
                if nsplit == 1 or n < 8:
                    return [(0, n)]
                h = n // 2
                return [(0, h), (h, n)]

            B1 = work.tile([128, 2, 34, 132], F16, tag="work")
            for (a, b) in bands(n1):
                nc.vector.tensor_add(B1[:, 0, a:b, :], D[:, 0, a:b, :],
                                     D[:, 1, a:b, :])
                nc.vector.tensor_add(B1[:, 1, a:b, :], D[:, 1, a:b, :],
                                     D[:, 0, a + 1:b + 1, :])
            B2 = work.tile([128, 2, 33, 132], F16, tag="work")
            for (a, b) in bands(33):
                nc.vector.tensor_add(B2[:, 0, a:b, :], B1[:, 0, a:b, :],
                                     B1[:, 1, a:b, :])
            for (a, b) in bands(n2):
                nc.vector.tensor_add(B2[:, 1, a:b, :], B1[:, 1, a:b, :],
                                     B1[:, 0, a + 1:b + 1, :])
            R = work.tile([128, 64, 132], F16, tag="work")
            for (a, b) in bands(32):
                nc.vector.tensor_add(R[:, 2 * a:2 * b:2, :], B2[:, 0, a:b, :],
                                     B2[:, 1, a:b, :])
                nc.vector.tensor_add(R[:, 2 * a + 1:2 * b:2, :],
                                     B2[:, 1, a:b, :], B2[:, 0, a + 1:b + 1, :])
            C1 = work.tile([128, 64, 132], F16, tag="work")
            for (a, b) in bands(64):
                nc.vector.tensor_add(C1[:, a:b, 0:65], R[:, a:b, 0:65],
                                     R[:, a:b, 66:131])
                nc.vector.tensor_add(C1[:, a:b, 66:131], R[:, a:b, 66:131],
                                     R[:, a:b, 1:66])
            C2 = work.tile([128, 64, 132], F16, tag="work")
            for (a, b) in bands(64):
                nc.vector.tensor_add(C2[:, a:b, 0:65], C1[:, a:b, 0:65],
                                     C1[:, a:b, 66:131])
                nc.vector.tensor_add(C2[:, a:b, 66:130], C1[:, a:b, 66:130],
                                     C1[:, a:b, 1:65])
            C3 = work.tile([128, 64, 132], F16, tag="work")
            for (a, b) in bands(64):
                nc.vector.tensor_add(C3[:, a:b, 0:64], C2[:, a:b, 0:64],
                                     C2[:, a:b, 66:130])
                nc.vector.tensor_add(C3[:, a:b, 66:130], C2[:, a:b, 66:130],
                                     C2[:, a:b, 1:65])
            return C3

        def emit_unit_epilogue(ct, half, C3):
            """noise+bias on Pool (in place), prelu+interleave+fp16 on ACT
            writing over the noise band tile, DMA out. 2 sub-bands."""
            for b in range(2):
                r0 = b * 32
                g0 = half * 64 + r0
                nyb = nypool.tile([128, 32, 128], F16, tag="ny")
                nc.sync.dma_start(nyb[:], nq[ct, :, g0:g0 + 32, :])
                for sg in range(2):
                    cs = slice(sg * 66, sg * 66 + 64)
                    nc.gpsimd.tensor_add(C3[:, r0:r0 + 32, cs],
                                         C3[:, r0:r0 + 32, cs],
                                         nyb[:, :, sg * 64:sg * 64 + 64])
                for sg in range(2):
                    cs = slice(sg * 66, sg * 66 + 64)
                    if epilogue == 'prelu':
                        nc.scalar.activation(
                            nyb[:, :, sg::2], C3[:, r0:r0 + 32, cs],
                            PRELU, bias=0.0, scale=1.0, alpha=0.2)
                    else:
                        nc.vector.scalar_tensor_tensor(
                            C3[:, r0:r0 + 32, cs], C3[:, r0:r0 + 32, cs], 0.2,
                            C3[:, r0:r0 + 32, cs],
                            mybir.AluOpType.mult, mybir.AluOpType.max)
                        nc.scalar.activation(
                            nyb[:, :, sg::2], C3[:, r0:r0 + 32, cs], COPY)
                nc.sync.dma_start(y[ct, :, g0:g0 + 32, :], nyb[:])

        # pre-allocate half-B D tiles (receive dup rows during half-A mms)
        dup_targets = {}
        units = [(0, 0), (0, 1), (1, 0), (1, 1)]
        pending = []     # (ct, half, C3) awaiting epilogue
        for (ct, half) in units:
            if half == 0:
                D = work.tile([128, 2, 36, 132], F16, tag="work")
                dup_targets[ct] = work.tile([128, 2, 34, 132], F16, tag="work", name=f"dupD{ct}")
            else:
                D = dup_targets[ct]
            emit_unit_mm(ct, half, D)
            C3 = emit_unit_fir(D, half, nsplit=2 if (ct, half) == (1, 1) else 1)
            pending.append((ct, half, C3))
            if len(pending) > 1:
                pct, ph, pC3 = pending.pop(0)
                emit_unit_epilogue(pct, ph, pC3)
        for (pct, ph, pC3) in pending:
            emit_unit_epilogue(pct, ph, pC3)
    if legalize:
        legalize_waits(nc)
    return nc


# ---------------------------------------------------------------------------
# Host-side preparation
# ---------------------------------------------------------------------------

def prep_inputs(x, weight, bias, noise_const, noise_strength):
    SQ2 = np.sqrt(2.0)
    w = np.asarray(weight, np.float64)
    inv = 1.0 / np.sqrt((w ** 2).sum(axis=(1, 2, 3)) + 1e-8)
    w = w * inv[:, None, None, None]
    wf = w[:, :, ::-1, ::-1] * (SQ2 / 16.0)       # fold FIR norm + lrelu gain
    # wq[cg, ci, u*3+v, co]
    wq = np.ascontiguousarray(
        wf.transpose(1, 2, 3, 0).reshape(4, 128, 9, 256), dtype=np.float16)

    n2 = np.asarray(noise_const, np.float64) * float(noise_strength) * SQ2
    # noise part: [r, s*64 + m] = n2[r, 2m+s]; plus per-partition bias per ct
    nflat = n2.reshape(128, 64, 2).transpose(0, 2, 1).reshape(128, 128)
    b2 = (np.asarray(bias, np.float64) * SQ2).reshape(2, 128)
    nq = np.ascontiguousarray(
        nflat[None, None, :, :] + b2[:, :, None, None], dtype=np.float16)

    x = np.asarray(x, np.float32)
    maps = []
    for bi in range(x.shape[0]):
        xp = np.zeros((512, 67, 67), np.float16)
        xp[:, 1:65, 1:65] = x[bi]
        maps.append({
            "xin": xp.reshape(4, 128, 67, 67),
            "wq": wq,
            "nq": nq,
        })
    return maps


_NC_CACHE = None


def kernel(x, weight, bias, noise_const, noise_strength):
    global _NC_CACHE
    if _NC_CACHE is None:
        _NC_CACHE = build_conv_nc()
    in_maps = prep_inputs(x, weight, bias, noise_const, noise_strength)
    res = run_bass_kernel_spmd(_NC_CACHE, in_maps, core_ids=list(range(8)))
    return np.ascontiguousarray(
        np.stack([r["y"].reshape(256, 128, 128) for r in res.results]),
        dtype=np.float32)


# revision 5
# speedup vs baseline: 1.2863x; 1.0525x over previous
"""Trainium2 Bass kernel for nn_Conv2d_61881888800824 (v2: box-cascade FIR).

StyleGAN2 synthesis layer: renorm(w) -> up2 (zero-insert) -> 4x4 FIR -> 3x3
conv -> +noise -> +bias -> lrelu(0.2)*sqrt(2).

v2 factorization: by conv associativity, y = f2 (*) (wf (*) up2(x)) with
f2 = outer([1,3,3,1])/16 separable AND [1,3,3,1] = [1,1]^(*3): the channel
contraction only needs wf's original 9 taps on the COARSE 64x64 grid (4x
fewer MACs than folding the FIR into the conv), and the FIR becomes six
box passes (pure adds) on cheap engines:

  D phases (rho,sigma in {e,o}^2), coarse grid, xp[a]=x[a-1] zero-padded:
    ee: wf[1,1]@xp[k,l]
    eo: wf[1,0]@xp[k,l]   + wf[1,2]@xp[k,l+1]
    oe: wf[0,1]@xp[k,l]   + wf[2,1]@xp[k+1,l]
    oo: wf[0,0]@xp[k,l] + wf[0,2]@xp[k,l+1] + wf[2,0]@xp[k+1,l] + wf[2,2]@xp[k+1,l+1]
  then per dim: B1e[k]=De[k]+Do[k]; B1o[k]=Do[k]+De[k+1];
                B2e[k]=B1e+B1o; B2o[k]=B1o[k]+B1e[k+1];
                ye[n]=B2e+B2o;  yo[n]=B2o[n]+B2e[n+1].

Engine split per core: PE = 720 fp16 matmuls (N=462); ACT = PSUM->SBUF fp16
copies + final Lrelu(+bias, col-interleave, fp16); DVE = 6 box passes (fp16
2x mode); Pool(GPSIMD) = noise add. 1/16 FIR gain and sqrt(2) folded into
weights/bias/noise host-side.

Sharding: data-parallel over batch - 1 image per NeuronCore, 8 cores.
"""
from contextlib import ExitStack

import numpy as np

import bass_rust
import concourse.bass as bass
import concourse.mybir as mybir
import concourse.tile as tile
from concourse.bass_utils import run_bass_kernel_spmd

F16 = mybir.dt.float16
F32 = mybir.dt.float32


# ---------------------------------------------------------------------------
# Wait legalization: this walrus build accepts at most ONE embedded sync wait
# per instruction. Tile can emit more (incl. same-engine self-waits that are
# provably satisfied by the engine's serial program order). Drop the provable
# ones; move the rest onto standalone EventSemaphore instructions inserted
# just before the over-limit instruction on the same engine.
# ---------------------------------------------------------------------------

def _is_async_update(inst) -> bool:
    n = type(inst).__name__
    return 'DMA' in n or 'Swdge' in n or 'Collective' in n or 'Dma' in n


def legalize_waits(nc, evsem_limit: int = 1) -> int:
    n_fixed = 0
    for fn in nc.m.functions:
        for bb in fn.blocks:
            insts = bb.instructions
            cum: dict[tuple, int] = {}
            out = []
            changed = False
            for inst in insts:
                si = inst.sync_info
                waits = list(si.on_wait) if si is not None and si.on_wait else []
                updates = list(si.on_update) if si is not None and si.on_update else []
                eng = inst.engine
                limit = 1
                if len(waits) > limit:
                    kept = []
                    for w in waits:
                        if (w.sync_type == 'semaphore'
                                and w.wait_mode == 'sem-ge-imm'
                                and w.wait_reg is None
                                and cum.get((eng, w.id), 0) >= w.wait_value):
                            continue
                        kept.append(w)
                    waits = kept
                if len(waits) > limit:
                    excess = waits[:-limit]
                    waits = waits[-limit:]
                    while excess:
                        take, excess = excess[:evsem_limit], excess[evsem_limit:]
                        ev = mybir.InstEventSemaphore(
                            name=nc.get_next_instruction_name(), ins=[], outs=[])
                        ev.engine = eng
                        ev.sync_info = bass_rust.SyncInfo(on_wait=take, on_update=[])
                        out.append(ev)
                    inst.sync_info = bass_rust.SyncInfo(on_wait=waits,
                                                        on_update=updates)
                    changed = True
                    n_fixed += 1
                elif si is not None and len(list(si.on_wait or [])) != len(waits):
                    inst.sync_info = bass_rust.SyncInfo(on_wait=waits,
                                                        on_update=updates)
                    changed = True
                    n_fixed += 1
                out.append(inst)
                if not _is_async_update(inst):
                    for u in updates:
                        if (u.sync_type == 'semaphore'
                                and u.update_mode == 'sem-inc'
                                and u.update_reg is None):
                            k = (eng, u.id)
                            cum[k] = cum.get(k, 0) + u.update_value
            if changed:
                bb.instructions = out
    return n_fixed


# ---------------------------------------------------------------------------
# Device kernel (per core: one batch image)
# ---------------------------------------------------------------------------

# phase (rho, sigma) -> [(u, v, dr, dc)] tap list
TAPS = {
    (0, 0): [(1, 1, 0, 0)],
    (0, 1): [(1, 0, 0, 0), (1, 2, 0, 1)],
    (1, 0): [(0, 1, 0, 0), (2, 1, 1, 0)],
    (1, 1): [(0, 0, 0, 0), (0, 2, 0, 1), (2, 0, 1, 0), (2, 2, 1, 1)],
}
CHUNKS = [(c * 7, min(7, 66 - c * 7)) for c in range(10)]   # (k0, nrows)
PRELU = mybir.ActivationFunctionType.Prelu
COPY = mybir.ActivationFunctionType.Copy


def build_conv_nc(legalize=True, epilogue='prelu'):
    nc = bass.Bass("TRN2", target_bir_lowering=False, debug=False)
    xin = nc.dram_tensor("xin", [4, 128, 67, 67], F16, kind="ExternalInput").ap()
    wq = nc.dram_tensor("wq", [4, 128, 9, 256], F16, kind="ExternalInput").ap()
    # nq[ct, p, r, s*64+m] = noise[r, 2m+s]*strength*sqrt2 + bias[ct*128+p]*sqrt2
    nq = nc.dram_tensor("nq", [2, 128, 128, 128], F16, kind="ExternalInput").ap()
    y = nc.dram_tensor("y", [2, 128, 128, 128], F16, kind="ExternalOutput").ap()

    with ExitStack() as ctx:
        tc = ctx.enter_context(tile.TileContext(nc))
        xpool = ctx.enter_context(tc.tile_pool(name="x", bufs=1))
        wpool = ctx.enter_context(tc.tile_pool(name="w", bufs=1))
        work = ctx.enter_context(tc.tile_pool(name="work", bufs=7))
        nypool = ctx.enter_context(tc.tile_pool(name="ny", bufs=2))
        pp = ctx.enter_context(tc.tile_pool(name="psum", bufs=8, space="PSUM"))

        wsb = wpool.tile([128, 9, 4, 256], F16)
        for cg in range(4):
            nc.sync.dma_start(wsb[:, :, cg, :], wq[cg])

        xq = xpool.tile([128, 4, 67, 67], F16)
        for (r0, r1) in ((0, 16), (16, 34), (34, 50), (50, 67)):
            for cg in range(4):
                nc.sync.dma_start(xq[:, cg, r0:r1, :], xin[cg, :, r0:r1, :])

        # 4 units = (ct, half). Half A: D rows 0..35 (local=global), y rows
        # 0..63. Half B: D rows 32..65 (local=global-32), y rows 64..127.
        # Rows 32..35 of D are copied into BOTH halves (from half A's k0=30
        # chunk) so the FIR never reads across unit tiles.
        def emit_unit_mm(ct, half, D):
            """Matmuls + PSUM->SBUF fp16 copies into the given D tile."""
            if half == 0:
                chunks = [(k0, 6) for k0 in range(0, 36, 6)]
                dbase = 0
            else:
                chunks = [(k0, 6) for k0 in range(36, 66, 6)]
                dbase = 32
            for (k0, nr) in chunks:
                for (rho, sg), taps in TAPS.items():
                    ps = pp.tile([128, 6, 66], F32, tag="psum")
                    nmm = len(taps) * 4
                    i = 0
                    for (u, v, dr, dc) in taps:
                        for cg in range(4):
                            nc.tensor.matmul(
                                ps[:, 0:nr, :],
                                wsb[:, u * 3 + v, cg, ct * 128:(ct + 1) * 128],
                                xq[:, cg, k0 + dr:k0 + dr + nr, dc:dc + 66],
                                start=(i == 0), stop=(i == nmm - 1))
                            i += 1
                    lk = k0 - dbase
                    nc.scalar.activation(
                        D[:, rho, lk:lk + nr, sg * 66:sg * 66 + 66],
                        ps[:, 0:nr, :], COPY)
                    if half == 0 and k0 == 30:
                        # dup rows 32..35 into the SAME ct's half-B D tile
                        nc.scalar.activation(
                            dup_targets[ct][:, rho, 0:4, sg * 66:sg * 66 + 66],
                            ps[:, 2:6, :], COPY)

        def emit_unit_fir(D, half, nsplit=1):
            """Box cascade for one unit. Returns C3 [128, 64, 132].

            nsplit=2 sub-bands each stage op so the cascade wavefronts
            behind this unit's own matmul stream (used for the last unit,
            whose chain otherwise runs entirely after PE finishes)."""
            n1 = 34 if half == 0 else 33       # B1 rows
            n2 = 33 if half == 0 else 32       # B2 rho1 rows

            def bands(n):
                if nsplit == 1 or n < 8:
                    return [(0, n)]
                h = n // 2
                return [(0, h), (h, n)]

            B1 = work.tile([128, 2, 34, 132], F16, tag="work")
            for (a, b) in bands(n1):
                nc.vector.tensor_add(B1[:, 0, a:b, :], D[:, 0, a:b, :],
                                     D[:, 1, a:b, :])
                nc.vector.tensor_add(B1[:, 1, a:b, :], D[:, 1, a:b, :],
                                     D[:, 0, a + 1:b + 1, :])
            B2 = work.tile([128, 2, 33, 132], F16, tag="work")
            for (a, b) in bands(33):
                nc.vector.tensor_add(B2[:, 0, a:b, :], B1[:, 0, a:b, :],
                                     B1[:, 1, a:b, :])
            for (a, b) in bands(n2):
                nc.vector.tensor_add(B2[:, 1, a:b, :], B1[:, 1, a:b, :],
                                     B1[:, 0, a + 1:b + 1, :])
            R = work.tile([128, 64, 132], F16, tag="work")
            for (a, b) in bands(32):
                nc.vector.tensor_add(R[:, 2 * a:2 * b:2, :], B2[:, 0, a:b, :],
                                     B2[:, 1, a:b, :])
                nc.vector.tensor_add(R[:, 2 * a + 1:2 * b:2, :],
                                     B2[:, 1, a:b, :], B2[:, 0, a + 1:b + 1, :])
            C1 = work.tile([128, 64, 132], F16, tag="work")
            for (a, b) in bands(64):
                nc.vector.tensor_add(C1[:, a:b, 0:65], R[:, a:b, 0:65],
                                     R[:, a:b, 66:131])
                nc.vector.tensor_add(C1[:, a:b, 66:131], R[:, a:b, 66:131],
                                     R[:, a:b, 1:66])
            C2 = work.tile([128, 64, 132], F16, tag="work")
            for (a, b) in bands(64):
                nc.vector.tensor_add(C2[:, a:b, 0:65], C1[:, a:b, 0:65],
                                     C1[:, a:b, 66:131])
                nc.vector.tensor_add(C2[:, a:b, 66:130], C1[:, a:b, 66:130],
                                     C1[:, a:b, 1:65])
            C3 = work.tile([128, 64, 132], F16, tag="work")
            for (a, b) in bands(64):
                nc.vector.tensor_add(C3[:, a:b, 0:64], C2[:, a:b, 0:64],
                                     C2[:, a:b, 66:130])
                nc.vector.tensor_add(C3[:, a:b, 66:130], C2[:, a:b, 66:130],
                                     C2[:, a:b, 1:65])
            return C3

        def emit_unit_epilogue(ct, half, C3):
            """noise+bias add in place on C3 (Pool normally; DVE for the
            final unit so the drain tail is short), prelu+interleave+fp16 on
            ACT writing over the noise band tile, DMA out. 2 sub-bands."""
            neng = nc.vector if (ct, half) == (1, 1) else nc.gpsimd
            for b in range(2):
                r0 = b * 32
                g0 = half * 64 + r0
                nyb = nypool.tile([128, 32, 128], F16, tag="ny")
                nc.sync.dma_start(nyb[:], nq[ct, :, g0:g0 + 32, :])
                for sg in range(2):
                    cs = slice(sg * 66, sg * 66 + 64)
                    neng.tensor_add(C3[:, r0:r0 + 32, cs],
                                    C3[:, r0:r0 + 32, cs],
                                    nyb[:, :, sg * 64:sg * 64 + 64])
                for sg in range(2):
                    cs = slice(sg * 66, sg * 66 + 64)
                    if epilogue == 'prelu':
                        nc.scalar.activation(
                            nyb[:, :, sg::2], C3[:, r0:r0 + 32, cs],
                            PRELU, bias=0.0, scale=1.0, alpha=0.2)
                    else:
                        nc.vector.scalar_tensor_tensor(
                            C3[:, r0:r0 + 32, cs], C3[:, r0:r0 + 32, cs], 0.2,
                            C3[:, r0:r0 + 32, cs],
                            mybir.AluOpType.mult, mybir.AluOpType.max)
                        nc.scalar.activation(
                            nyb[:, :, sg::2], C3[:, r0:r0 + 32, cs], COPY)
                nc.sync.dma_start(y[ct, :, g0:g0 + 32, :], nyb[:])

        # pre-allocate half-B D tiles (receive dup rows during half-A mms)
        dup_targets = {}
        units = [(0, 0), (0, 1), (1, 0), (1, 1)]
        pending = []     # (ct, half, C3) awaiting epilogue
        for (ct, half) in units:
            if half == 0:
                D = work.tile([128, 2, 36, 132], F16, tag="work")
                dup_targets[ct] = work.tile([128, 2, 34, 132], F16, tag="work", name=f"dupD{ct}")
            else:
                D = dup_targets[ct]
            emit_unit_mm(ct, half, D)
            C3 = emit_unit_fir(D, half, nsplit=2)
            pending.append((ct, half, C3))
            if len(pending) > 1:
                pct, ph, pC3 = pending.pop(0)
                emit_unit_epilogue(pct, ph, pC3)
        for (pct, ph, pC3) in pending:
            emit_unit_epilogue(pct, ph, pC3)
    if legalize:
        legalize_waits(nc)
    return nc


# ---------------------------------------------------------------------------
# Host-side preparation
# ---------------------------------------------------------------------------

def prep_inputs(x, weight, bias, noise_const, noise_strength):
    SQ2 = np.sqrt(2.0)
    w = np.asarray(weight, np.float64)
    inv = 1.0 / np.sqrt((w ** 2).sum(axis=(1, 2, 3)) + 1e-8)
    w = w * inv[:, None, None, None]
    wf = w[:, :, ::-1, ::-1] * (SQ2 / 16.0)       # fold FIR norm + lrelu gain
    # wq[cg, ci, u*3+v, co]
    wq = np.ascontiguousarray(
        wf.transpose(1, 2, 3, 0).reshape(4, 128, 9, 256), dtype=np.float16)

    n2 = np.asarray(noise_const, np.float64) * float(noise_strength) * SQ2
    # noise part: [r, s*64 + m] = n2[r, 2m+s]; plus per-partition bias per ct
    nflat = n2.reshape(128, 64, 2).transpose(0, 2, 1).reshape(128, 128)
    b2 = (np.asarray(bias, np.float64) * SQ2).reshape(2, 128)
    nq = np.ascontiguousarray(
        nflat[None, None, :, :] + b2[:, :, None, None], dtype=np.float16)

    x = np.asarray(x, np.float32)
    maps = []
    for bi in range(x.shape[0]):
        xp = np.zeros((512, 67, 67), np.float16)
        xp[:, 1:65, 1:65] = x[bi]
        maps.append({
            "xin": xp.reshape(4, 128, 67, 67),
            "wq": wq,
            "nq": nq,
        })
    return maps


_NC_CACHE = None


def kernel(x, weight, bias, noise_const, noise_strength):
    global _NC_CACHE
    if _NC_CACHE is None:
        _NC_CACHE = build_conv_nc()
    in_maps = prep_inputs(x, weight, bias, noise_const, noise_strength)
    res = run_bass_kernel_spmd(_NC_CACHE, in_maps, core_ids=list(range(8)))
    return np.ascontiguousarray(
        np.stack([r["y"].reshape(256, 128, 128) for r in res.results]),
        dtype=np.float32)


# revision 6
# speedup vs baseline: 1.3780x; 1.0714x over previous
"""Trainium2 Bass kernel for nn_Conv2d_61881888800824 (v2: box-cascade FIR).

StyleGAN2 synthesis layer: renorm(w) -> up2 (zero-insert) -> 4x4 FIR -> 3x3
conv -> +noise -> +bias -> lrelu(0.2)*sqrt(2).

v2 factorization: by conv associativity, y = f2 (*) (wf (*) up2(x)) with
f2 = outer([1,3,3,1])/16 separable AND [1,3,3,1] = [1,1]^(*3): the channel
contraction only needs wf's original 9 taps on the COARSE 64x64 grid (4x
fewer MACs than folding the FIR into the conv), and the FIR becomes six
box passes (pure adds) on cheap engines:

  D phases (rho,sigma in {e,o}^2), coarse grid, xp[a]=x[a-1] zero-padded:
    ee: wf[1,1]@xp[k,l]
    eo: wf[1,0]@xp[k,l]   + wf[1,2]@xp[k,l+1]
    oe: wf[0,1]@xp[k,l]   + wf[2,1]@xp[k+1,l]
    oo: wf[0,0]@xp[k,l] + wf[0,2]@xp[k,l+1] + wf[2,0]@xp[k+1,l] + wf[2,2]@xp[k+1,l+1]
  then per dim: B1e[k]=De[k]+Do[k]; B1o[k]=Do[k]+De[k+1];
                B2e[k]=B1e+B1o; B2o[k]=B1o[k]+B1e[k+1];
                ye[n]=B2e+B2o;  yo[n]=B2o[n]+B2e[n+1].

Engine split per core: PE = 720 fp16 matmuls (N=462); ACT = PSUM->SBUF fp16
copies + final Lrelu(+bias, col-interleave, fp16); DVE = 6 box passes (fp16
2x mode); Pool(GPSIMD) = noise add. 1/16 FIR gain and sqrt(2) folded into
weights/bias/noise host-side.

Sharding: data-parallel over batch - 1 image per NeuronCore, 8 cores.
"""
from contextlib import ExitStack

import numpy as np

import bass_rust
import concourse.bass as bass
import concourse.mybir as mybir
import concourse.tile as tile
from concourse.bass_utils import run_bass_kernel_spmd

F16 = mybir.dt.float16
F32 = mybir.dt.float32


# ---------------------------------------------------------------------------
# Wait legalization: this walrus build accepts at most ONE embedded sync wait
# per instruction. Tile can emit more (incl. same-engine self-waits that are
# provably satisfied by the engine's serial program order). Drop the provable
# ones; move the rest onto standalone EventSemaphore instructions inserted
# just before the over-limit instruction on the same engine.
# ---------------------------------------------------------------------------

def _is_async_update(inst) -> bool:
    n = type(inst).__name__
    return 'DMA' in n or 'Swdge' in n or 'Collective' in n or 'Dma' in n


def legalize_waits(nc, evsem_limit: int = 1) -> int:
    n_fixed = 0
    for fn in nc.m.functions:
        for bb in fn.blocks:
            insts = bb.instructions
            cum: dict[tuple, int] = {}
            out = []
            changed = False
            for inst in insts:
                si = inst.sync_info
                waits = list(si.on_wait) if si is not None and si.on_wait else []
                updates = list(si.on_update) if si is not None and si.on_update else []
                eng = inst.engine
                limit = 1
                if len(waits) > limit:
                    kept = []
                    for w in waits:
                        if (w.sync_type == 'semaphore'
                                and w.wait_mode == 'sem-ge-imm'
                                and w.wait_reg is None
                                and cum.get((eng, w.id), 0) >= w.wait_value):
                            continue
                        kept.append(w)
                    waits = kept
                if len(waits) > limit:
                    excess = waits[:-limit]
                    waits = waits[-limit:]
                    while excess:
                        take, excess = excess[:evsem_limit], excess[evsem_limit:]
                        ev = mybir.InstEventSemaphore(
                            name=nc.get_next_instruction_name(), ins=[], outs=[])
                        ev.engine = eng
                        ev.sync_info = bass_rust.SyncInfo(on_wait=take, on_update=[])
                        out.append(ev)
                    inst.sync_info = bass_rust.SyncInfo(on_wait=waits,
                                                        on_update=updates)
                    changed = True
                    n_fixed += 1
                elif si is not None and len(list(si.on_wait or [])) != len(waits):
                    inst.sync_info = bass_rust.SyncInfo(on_wait=waits,
                                                        on_update=updates)
                    changed = True
                    n_fixed += 1
                out.append(inst)
                if not _is_async_update(inst):
                    for u in updates:
                        if (u.sync_type == 'semaphore'
                                and u.update_mode == 'sem-inc'
                                and u.update_reg is None):
                            k = (eng, u.id)
                            cum[k] = cum.get(k, 0) + u.update_value
            if changed:
                bb.instructions = out
    return n_fixed


# ---------------------------------------------------------------------------
# Device kernel (per core: one batch image)
# ---------------------------------------------------------------------------

# phase (rho, sigma) -> [(u, v, dr, dc)] tap list
TAPS = {
    (0, 0): [(1, 1, 0, 0)],
    (0, 1): [(1, 0, 0, 0), (1, 2, 0, 1)],
    (1, 0): [(0, 1, 0, 0), (2, 1, 1, 0)],
    (1, 1): [(0, 0, 0, 0), (0, 2, 0, 1), (2, 0, 1, 0), (2, 2, 1, 1)],
}
CHUNKS = [(c * 7, min(7, 66 - c * 7)) for c in range(10)]   # (k0, nrows)
PRELU = mybir.ActivationFunctionType.Prelu
COPY = mybir.ActivationFunctionType.Copy


def build_conv_nc(legalize=True, epilogue='prelu'):
    nc = bass.Bass("TRN2", target_bir_lowering=False, debug=False)
    xin = nc.dram_tensor("xin", [4, 128, 67, 67], F16, kind="ExternalInput").ap()
    wq = nc.dram_tensor("wq", [4, 128, 9, 256], F16, kind="ExternalInput").ap()
    # nq[ct, p, r, s*64+m] = noise[r, 2m+s]*strength*sqrt2 + bias[ct*128+p]*sqrt2
    nq = nc.dram_tensor("nq", [2, 128, 128, 128], F16, kind="ExternalInput").ap()
    y = nc.dram_tensor("y", [2, 128, 128, 128], F16, kind="ExternalOutput").ap()

    with ExitStack() as ctx:
        tc = ctx.enter_context(tile.TileContext(nc))
        xpool = ctx.enter_context(tc.tile_pool(name="x", bufs=1))
        wpool = ctx.enter_context(tc.tile_pool(name="w", bufs=1))
        work = ctx.enter_context(tc.tile_pool(name="work", bufs=7))
        nypool = ctx.enter_context(tc.tile_pool(name="ny", bufs=2))
        pp = ctx.enter_context(tc.tile_pool(name="psum", bufs=8, space="PSUM"))

        wsb = wpool.tile([128, 9, 4, 256], F16)
        for cg in range(4):
            nc.sync.dma_start(wsb[:, :, cg, :], wq[cg])

        xq = xpool.tile([128, 4, 67, 67], F16)
        for (r0, r1) in ((0, 16), (16, 34), (34, 50), (50, 67)):
            for cg in range(4):
                nc.sync.dma_start(xq[:, cg, r0:r1, :], xin[cg, :, r0:r1, :])

        # 4 units = (ct, half). Half A: D rows 0..35 (local=global), y rows
        # 0..63. Half B: D rows 32..65 (local=global-32), y rows 64..127.
        # Rows 32..35 of D are copied into BOTH halves (from half A's k0=30
        # chunk) so the FIR never reads across unit tiles.
        def emit_unit_mm(ct, half, D):
            """Matmuls + PSUM->SBUF fp16 copies into the given D tile."""
            if half == 0:
                chunks = [(k0, 6) for k0 in range(0, 36, 6)]
                dbase = 0
            else:
                chunks = [(k0, 6) for k0 in range(36, 66, 6)]
                dbase = 32
            for (k0, nr) in chunks:
                for (rho, sg), taps in TAPS.items():
                    ps = pp.tile([128, 6, 66], F32, tag="psum")
                    nmm = len(taps) * 4
                    i = 0
                    for (u, v, dr, dc) in taps:
                        for cg in range(4):
                            nc.tensor.matmul(
                                ps[:, 0:nr, :],
                                wsb[:, u * 3 + v, cg, ct * 128:(ct + 1) * 128],
                                xq[:, cg, k0 + dr:k0 + dr + nr, dc:dc + 66],
                                start=(i == 0), stop=(i == nmm - 1))
                            i += 1
                    lk = k0 - dbase
                    nc.scalar.activation(
                        D[:, rho, lk:lk + nr, sg * 66:sg * 66 + 66],
                        ps[:, 0:nr, :], COPY)
                    if half == 0 and k0 == 30:
                        # dup rows 32..35 into the SAME ct's half-B D tile
                        nc.scalar.activation(
                            dup_targets[ct][:, rho, 0:4, sg * 66:sg * 66 + 66],
                            ps[:, 2:6, :], COPY)

        def emit_unit_fir(D, half, nsplit=1):
            """Box cascade for one unit. Returns C3 [128, 64, 132].

            nsplit=2 sub-bands each stage op so the cascade wavefronts
            behind this unit's own matmul stream (used for the last unit,
            whose chain otherwise runs entirely after PE finishes)."""
            n1 = 34 if half == 0 else 33       # B1 rows
            n2 = 33 if half == 0 else 32       # B2 rho1 rows

            def bands(n):
                if nsplit == 1 or n < 8:
                    return [(0, n)]
                h = n // 2
                return [(0, h), (h, n)]

            B1 = work.tile([128, 2, 34, 132], F16, tag="work")
            for (a, b) in bands(n1):
                nc.vector.tensor_add(B1[:, 0, a:b, :], D[:, 0, a:b, :],
                                     D[:, 1, a:b, :])
                nc.vector.tensor_add(B1[:, 1, a:b, :], D[:, 1, a:b, :],
                                     D[:, 0, a + 1:b + 1, :])
            B2 = work.tile([128, 2, 33, 132], F16, tag="work")
            for (a, b) in bands(33):
                nc.vector.tensor_add(B2[:, 0, a:b, :], B1[:, 0, a:b, :],
                                     B1[:, 1, a:b, :])
            for (a, b) in bands(n2):
                nc.vector.tensor_add(B2[:, 1, a:b, :], B1[:, 1, a:b, :],
                                     B1[:, 0, a + 1:b + 1, :])
            R = work.tile([128, 64, 132], F16, tag="work")
            for (a, b) in bands(32):
                nc.vector.tensor_add(R[:, 2 * a:2 * b:2, :], B2[:, 0, a:b, :],
                                     B2[:, 1, a:b, :])
                nc.vector.tensor_add(R[:, 2 * a + 1:2 * b:2, :],
                                     B2[:, 1, a:b, :], B2[:, 0, a + 1:b + 1, :])
            C1 = work.tile([128, 64, 132], F16, tag="work")
            for (a, b) in bands(64):
                nc.vector.tensor_add(C1[:, a:b, 0:65], R[:, a:b, 0:65],
                                     R[:, a:b, 66:131])
                nc.vector.tensor_add(C1[:, a:b, 66:131], R[:, a:b, 66:131],
                                     R[:, a:b, 1:66])
            C2 = work.tile([128, 64, 132], F16, tag="work")
            for (a, b) in bands(64):
                nc.vector.tensor_add(C2[:, a:b, 0:65], C1[:, a:b, 0:65],
                                     C1[:, a:b, 66:131])
                nc.vector.tensor_add(C2[:, a:b, 66:130], C1[:, a:b, 66:130],
                                     C1[:, a:b, 1:65])
            C3 = work.tile([128, 64, 132], F16, tag="work")
            for (a, b) in bands(64):
                nc.vector.tensor_add(C3[:, a:b, 0:64], C2[:, a:b, 0:64],
                                     C2[:, a:b, 66:130])
                nc.vector.tensor_add(C3[:, a:b, 66:130], C2[:, a:b, 66:130],
                                     C2[:, a:b, 1:65])
            return C3

        def emit_unit_epilogue(ct, half, C3):
            """noise+bias add in place on C3 (Pool normally; DVE for the
            final unit so the drain tail is short), prelu+interleave+fp16 on
            ACT writing over the noise band tile, DMA out. 2 sub-bands."""
            neng = nc.vector if ct == 1 else nc.gpsimd
            for b in range(2):
                r0 = b * 32
                g0 = half * 64 + r0
                nyb = nypool.tile([128, 32, 128], F16, tag="ny")
                nc.sync.dma_start(nyb[:], nq[ct, :, g0:g0 + 32, :])
                for sg in range(2):
                    cs = slice(sg * 66, sg * 66 + 64)
                    neng.tensor_add(C3[:, r0:r0 + 32, cs],
                                    C3[:, r0:r0 + 32, cs],
                                    nyb[:, :, sg * 64:sg * 64 + 64])
                for sg in range(2):
                    cs = slice(sg * 66, sg * 66 + 64)
                    if epilogue == 'prelu':
                        nc.scalar.activation(
                            nyb[:, :, sg::2], C3[:, r0:r0 + 32, cs],
                            PRELU, bias=0.0, scale=1.0, alpha=0.2)
                    else:
                        nc.vector.scalar_tensor_tensor(
                            C3[:, r0:r0 + 32, cs], C3[:, r0:r0 + 32, cs], 0.2,
                            C3[:, r0:r0 + 32, cs],
                            mybir.AluOpType.mult, mybir.AluOpType.max)
                        nc.scalar.activation(
                            nyb[:, :, sg::2], C3[:, r0:r0 + 32, cs], COPY)
                nc.sync.dma_start(y[ct, :, g0:g0 + 32, :], nyb[:])

        # pre-allocate half-B D tiles (receive dup rows during half-A mms)
        dup_targets = {}
        units = [(0, 0), (0, 1), (1, 0), (1, 1)]
        pending = []     # (ct, half, C3) awaiting epilogue
        for (ct, half) in units:
            if half == 0:
                D = work.tile([128, 2, 36, 132], F16, tag="work")
                dup_targets[ct] = work.tile([128, 2, 34, 132], F16, tag="work", name=f"dupD{ct}")
            else:
                D = dup_targets[ct]
            emit_unit_mm(ct, half, D)
            if pending:
                pct, ph, pC3 = pending.pop(0)
                emit_unit_epilogue(pct, ph, pC3)
            C3 = emit_unit_fir(D, half, nsplit=2)
            pending.append((ct, half, C3))
        for (pct, ph, pC3) in pending:
            emit_unit_epilogue(pct, ph, pC3)
    if legalize:
        legalize_waits(nc)
    return nc


# ---------------------------------------------------------------------------
# Host-side preparation
# ---------------------------------------------------------------------------

def prep_inputs(x, weight, bias, noise_const, noise_strength):
    SQ2 = np.sqrt(2.0)
    w = np.asarray(weight, np.float64)
    inv = 1.0 / np.sqrt((w ** 2).sum(axis=(1, 2, 3)) + 1e-8)
    w = w * inv[:, None, None, None]
    wf = w[:, :, ::-1, ::-1] * (SQ2 / 16.0)       # fold FIR norm + lrelu gain
    # wq[cg, ci, u*3+v, co]
    wq = np.ascontiguousarray(
        wf.transpose(1, 2, 3, 0).reshape(4, 128, 9, 256), dtype=np.float16)

    n2 = np.asarray(noise_const, np.float64) * float(noise_strength) * SQ2
    # noise part: [r, s*64 + m] = n2[r, 2m+s]; plus per-partition bias per ct
    nflat = n2.reshape(128, 64, 2).transpose(0, 2, 1).reshape(128, 128)
    b2 = (np.asarray(bias, np.float64) * SQ2).reshape(2, 128)
    nq = np.ascontiguousarray(
        nflat[None, None, :, :] + b2[:, :, None, None], dtype=np.float16)

    x = np.asarray(x, np.float32)
    maps = []
    for bi in range(x.shape[0]):
        xp = np.zeros((512, 67, 67), np.float16)
        xp[:, 1:65, 1:65] = x[bi]
        maps.append({
            "xin": xp.reshape(4, 128, 67, 67),
            "wq": wq,
            "nq": nq,
        })
    return maps


_NC_CACHE = None


def kernel(x, weight, bias, noise_const, noise_strength):
    global _NC_CACHE
    if _NC_CACHE is None:
        _NC_CACHE = build_conv_nc()
    in_maps = prep_inputs(x, weight, bias, noise_const, noise_strength)
    res = run_bass_kernel_spmd(_NC_CACHE, in_maps, core_ids=list(range(8)))
    return np.ascontiguousarray(
        np.stack([r["y"].reshape(256, 128, 128) for r in res.results]),
        dtype=np.float32)


# revision 7
# speedup vs baseline: 1.3931x; 1.0109x over previous
"""Trainium2 Bass kernel for nn_Conv2d_61881888800824 (v2: box-cascade FIR).

StyleGAN2 synthesis layer: renorm(w) -> up2 (zero-insert) -> 4x4 FIR -> 3x3
conv -> +noise -> +bias -> lrelu(0.2)*sqrt(2).

v2 factorization: by conv associativity, y = f2 (*) (wf (*) up2(x)) with
f2 = outer([1,3,3,1])/16 separable AND [1,3,3,1] = [1,1]^(*3): the channel
contraction only needs wf's original 9 taps on the COARSE 64x64 grid (4x
fewer MACs than folding the FIR into the conv), and the FIR becomes six
box passes (pure adds) on cheap engines:

  D phases (rho,sigma in {e,o}^2), coarse grid, xp[a]=x[a-1] zero-padded:
    ee: wf[1,1]@xp[k,l]
    eo: wf[1,0]@xp[k,l]   + wf[1,2]@xp[k,l+1]
    oe: wf[0,1]@xp[k,l]   + wf[2,1]@xp[k+1,l]
    oo: wf[0,0]@xp[k,l] + wf[0,2]@xp[k,l+1] + wf[2,0]@xp[k+1,l] + wf[2,2]@xp[k+1,l+1]
  then per dim: B1e[k]=De[k]+Do[k]; B1o[k]=Do[k]+De[k+1];
                B2e[k]=B1e+B1o; B2o[k]=B1o[k]+B1e[k+1];
                ye[n]=B2e+B2o;  yo[n]=B2o[n]+B2e[n+1].

Engine split per core: PE = 720 fp16 matmuls (N=462); ACT = PSUM->SBUF fp16
copies + final Lrelu(+bias, col-interleave, fp16); DVE = 6 box passes (fp16
2x mode); Pool(GPSIMD) = noise add. 1/16 FIR gain and sqrt(2) folded into
weights/bias/noise host-side.

Sharding: data-parallel over batch - 1 image per NeuronCore, 8 cores.
"""
from contextlib import ExitStack

import numpy as np

import bass_rust
import concourse.bass as bass
import concourse.mybir as mybir
import concourse.tile as tile
from concourse.bass_utils import run_bass_kernel_spmd

F16 = mybir.dt.float16
F32 = mybir.dt.float32


# ---------------------------------------------------------------------------
# Wait legalization: this walrus build accepts at most ONE embedded sync wait
# per instruction. Tile can emit more (incl. same-engine self-waits that are
# provably satisfied by the engine's serial program order). Drop the provable
# ones; move the rest onto standalone EventSemaphore instructions inserted
# just before the over-limit instruction on the same engine.
# ---------------------------------------------------------------------------

def _is_async_update(inst) -> bool:
    n = type(inst).__name__
    return 'DMA' in n or 'Swdge' in n or 'Collective' in n or 'Dma' in n


def legalize_waits(nc, evsem_limit: int = 1) -> int:
    n_fixed = 0
    for fn in nc.m.functions:
        for bb in fn.blocks:
            insts = bb.instructions
            cum: dict[tuple, int] = {}
            out = []
            changed = False
            for inst in insts:
                si = inst.sync_info
                waits = list(si.on_wait) if si is not None and si.on_wait else []
                updates = list(si.on_update) if si is not None and si.on_update else []
                eng = inst.engine
                limit = 1
                if len(waits) > limit:
                    kept = []
                    for w in waits:
                        if (w.sync_type == 'semaphore'
                                and w.wait_mode == 'sem-ge-imm'
                                and w.wait_reg is None
                                and cum.get((eng, w.id), 0) >= w.wait_value):
                            continue
                        kept.append(w)
                    waits = kept
                if len(waits) > limit:
                    excess = waits[:-limit]
                    waits = waits[-limit:]
                    while excess:
                        take, excess = excess[:evsem_limit], excess[evsem_limit:]
                        ev = mybir.InstEventSemaphore(
                            name=nc.get_next_instruction_name(), ins=[], outs=[])
                        ev.engine = eng
                        ev.sync_info = bass_rust.SyncInfo(on_wait=take, on_update=[])
                        out.append(ev)
                    inst.sync_info = bass_rust.SyncInfo(on_wait=waits,
                                                        on_update=updates)
                    changed = True
                    n_fixed += 1
                elif si is not None and len(list(si.on_wait or [])) != len(waits):
                    inst.sync_info = bass_rust.SyncInfo(on_wait=waits,
                                                        on_update=updates)
                    changed = True
                    n_fixed += 1
                out.append(inst)
                if not _is_async_update(inst):
                    for u in updates:
                        if (u.sync_type == 'semaphore'
                                and u.update_mode == 'sem-inc'
                                and u.update_reg is None):
                            k = (eng, u.id)
                            cum[k] = cum.get(k, 0) + u.update_value
            if changed:
                bb.instructions = out
    return n_fixed


# ---------------------------------------------------------------------------
# Device kernel (per core: one batch image)
# ---------------------------------------------------------------------------

# phase (rho, sigma) -> [(u, v, dr, dc)] tap list
TAPS = {
    (0, 0): [(1, 1, 0, 0)],
    (0, 1): [(1, 0, 0, 0), (1, 2, 0, 1)],
    (1, 0): [(0, 1, 0, 0), (2, 1, 1, 0)],
    (1, 1): [(0, 0, 0, 0), (0, 2, 0, 1), (2, 0, 1, 0), (2, 2, 1, 1)],
}
CHUNKS = [(c * 7, min(7, 66 - c * 7)) for c in range(10)]   # (k0, nrows)
PRELU = mybir.ActivationFunctionType.Prelu
COPY = mybir.ActivationFunctionType.Copy


def build_conv_nc(legalize=True, epilogue='prelu'):
    nc = bass.Bass("TRN2", target_bir_lowering=False, debug=False)
    xin = nc.dram_tensor("xin", [4, 128, 67, 67], F16, kind="ExternalInput").ap()
    wq = nc.dram_tensor("wq", [4, 128, 9, 256], F16, kind="ExternalInput").ap()
    # nq[ct, p, r, s*64+m] = noise[r, 2m+s]*strength*sqrt2 + bias[ct*128+p]*sqrt2
    nq = nc.dram_tensor("nq", [2, 128, 128, 128], F16, kind="ExternalInput").ap()
    y = nc.dram_tensor("y", [2, 128, 128, 128], F16, kind="ExternalOutput").ap()

    with ExitStack() as ctx:
        tc = ctx.enter_context(tile.TileContext(nc))
        xpool = ctx.enter_context(tc.tile_pool(name="x", bufs=1))
        wpool = ctx.enter_context(tc.tile_pool(name="w", bufs=1))
        work = ctx.enter_context(tc.tile_pool(name="work", bufs=7))
        nypool = ctx.enter_context(tc.tile_pool(name="ny", bufs=2))
        pp = ctx.enter_context(tc.tile_pool(name="psum", bufs=8, space="PSUM"))

        wsb = wpool.tile([128, 9, 4, 256], F16)
        for cg in range(4):
            nc.sync.dma_start(wsb[:, :, cg, :], wq[cg])

        xq = xpool.tile([128, 4, 67, 67], F16)
        for (r0, r1) in ((0, 16), (16, 34), (34, 50), (50, 67)):
            for cg in range(4):
                nc.sync.dma_start(xq[:, cg, r0:r1, :], xin[cg, :, r0:r1, :])

        # 4 units = (ct, half). Half A: D rows 0..35 (local=global), y rows
        # 0..63. Half B: D rows 32..65 (local=global-32), y rows 64..127.
        # Rows 32..35 of D are copied into BOTH halves (from half A's k0=30
        # chunk) so the FIR never reads across unit tiles.
        def emit_unit_mm(ct, half, D):
            """Matmuls + PSUM->SBUF fp16 copies into the given D tile."""
            if half == 0:
                chunks = [(k0, 6) for k0 in range(0, 36, 6)]
                dbase = 0
            else:
                chunks = [(k0, 6) for k0 in range(36, 66, 6)]
                dbase = 32
            for (k0, nr) in chunks:
                for (rho, sg), taps in TAPS.items():
                    ps = pp.tile([128, 6, 66], F32, tag="psum")
                    nmm = len(taps) * 4
                    i = 0
                    for (u, v, dr, dc) in taps:
                        for cg in range(4):
                            nc.tensor.matmul(
                                ps[:, 0:nr, :],
                                wsb[:, u * 3 + v, cg, ct * 128:(ct + 1) * 128],
                                xq[:, cg, k0 + dr:k0 + dr + nr, dc:dc + 66],
                                start=(i == 0), stop=(i == nmm - 1))
                            i += 1
                    lk = k0 - dbase
                    nc.scalar.activation(
                        D[:, rho, lk:lk + nr, sg * 66:sg * 66 + 66],
                        ps[:, 0:nr, :], COPY)
                    if half == 0 and k0 == 30:
                        # dup rows 32..35 into the SAME ct's half-B D tile
                        nc.scalar.activation(
                            dup_targets[ct][:, rho, 0:4, sg * 66:sg * 66 + 66],
                            ps[:, 2:6, :], COPY)

        def emit_unit_fir(D, half, nsplit=1):
            """Box cascade for one unit. Returns C3 [128, 64, 132].

            nsplit=2 sub-bands each stage op so the cascade wavefronts
            behind this unit's own matmul stream (used for the last unit,
            whose chain otherwise runs entirely after PE finishes)."""
            n1 = 34 if half == 0 else 33       # B1 rows
            n2 = 33 if half == 0 else 32       # B2 rho1 rows

            def bands(n):
                if nsplit == 1 or n < 8:
                    return [(0, n)]
                h = n // 2
                return [(0, h), (h, n)]

            B1 = work.tile([128, 2, 34, 132], F16, tag="work")
            for (a, b) in bands(n1):
                nc.vector.tensor_add(B1[:, 0, a:b, :], D[:, 0, a:b, :],
                                     D[:, 1, a:b, :])
                nc.vector.tensor_add(B1[:, 1, a:b, :], D[:, 1, a:b, :],
                                     D[:, 0, a + 1:b + 1, :])
            B2 = work.tile([128, 2, 33, 132], F16, tag="work")
            for (a, b) in bands(33):
                nc.vector.tensor_add(B2[:, 0, a:b, :], B1[:, 0, a:b, :],
                                     B1[:, 1, a:b, :])
            for (a, b) in bands(n2):
                nc.vector.tensor_add(B2[:, 1, a:b, :], B1[:, 1, a:b, :],
                                     B1[:, 0, a + 1:b + 1, :])
            R = work.tile([128, 64, 132], F16, tag="work")
            for (a, b) in bands(32):
                nc.vector.tensor_add(R[:, 2 * a:2 * b:2, :], B2[:, 0, a:b, :],
                                     B2[:, 1, a:b, :])
                nc.vector.tensor_add(R[:, 2 * a + 1:2 * b:2, :],
                                     B2[:, 1, a:b, :], B2[:, 0, a + 1:b + 1, :])
            C1 = work.tile([128, 64, 132], F16, tag="work")
            for (a, b) in bands(64):
                nc.vector.tensor_add(C1[:, a:b, 0:65], R[:, a:b, 0:65],
                                     R[:, a:b, 66:131])
                nc.vector.tensor_add(C1[:, a:b, 66:131], R[:, a:b, 66:131],
                                     R[:, a:b, 1:66])
            C2 = work.tile([128, 64, 132], F16, tag="work")
            for (a, b) in bands(64):
                nc.vector.tensor_add(C2[:, a:b, 0:65], C1[:, a:b, 0:65],
                                     C1[:, a:b, 66:131])
                nc.vector.tensor_add(C2[:, a:b, 66:130], C1[:, a:b, 66:130],
                                     C1[:, a:b, 1:65])
            C3 = work.tile([128, 64, 132], F16, tag="work")
            for (a, b) in bands(64):
                nc.vector.tensor_add(C3[:, a:b, 0:64], C2[:, a:b, 0:64],
                                     C2[:, a:b, 66:130])
                nc.vector.tensor_add(C3[:, a:b, 66:130], C2[:, a:b, 66:130],
                                     C2[:, a:b, 1:65])
            return C3

        def emit_unit_epilogue(ct, half, C3):
            """noise+bias add in place on C3 (Pool normally; DVE for the
            final unit so the drain tail is short), prelu+interleave+fp16 on
            ACT writing over the noise band tile, DMA out. 2 sub-bands."""
            neng = nc.vector
            for b in range(2):
                r0 = b * 32
                g0 = half * 64 + r0
                nyb = nypool.tile([128, 32, 128], F16, tag="ny")
                nc.sync.dma_start(nyb[:], nq[ct, :, g0:g0 + 32, :])
                for sg in range(2):
                    cs = slice(sg * 66, sg * 66 + 64)
                    neng.tensor_add(C3[:, r0:r0 + 32, cs],
                                    C3[:, r0:r0 + 32, cs],
                                    nyb[:, :, sg * 64:sg * 64 + 64])
                for sg in range(2):
                    cs = slice(sg * 66, sg * 66 + 64)
                    if epilogue == 'prelu':
                        nc.scalar.activation(
                            nyb[:, :, sg::2], C3[:, r0:r0 + 32, cs],
                            PRELU, bias=0.0, scale=1.0, alpha=0.2)
                    else:
                        nc.vector.scalar_tensor_tensor(
                            C3[:, r0:r0 + 32, cs], C3[:, r0:r0 + 32, cs], 0.2,
                            C3[:, r0:r0 + 32, cs],
                            mybir.AluOpType.mult, mybir.AluOpType.max)
                        nc.scalar.activation(
                            nyb[:, :, sg::2], C3[:, r0:r0 + 32, cs], COPY)
                nc.sync.dma_start(y[ct, :, g0:g0 + 32, :], nyb[:])

        # pre-allocate half-B D tiles (receive dup rows during half-A mms)
        dup_targets = {}
        units = [(0, 0), (0, 1), (1, 0), (1, 1)]
        pending = []     # (ct, half, C3) awaiting epilogue
        for (ct, half) in units:
            if half == 0:
                D = work.tile([128, 2, 36, 132], F16, tag="work")
                dup_targets[ct] = work.tile([128, 2, 34, 132], F16, tag="work", name=f"dupD{ct}")
            else:
                D = dup_targets[ct]
            emit_unit_mm(ct, half, D)
            if pending:
                pct, ph, pC3 = pending.pop(0)
                emit_unit_epilogue(pct, ph, pC3)
            C3 = emit_unit_fir(D, half, nsplit=2)
            pending.append((ct, half, C3))
        for (pct, ph, pC3) in pending:
            emit_unit_epilogue(pct, ph, pC3)
    if legalize:
        legalize_waits(nc)
    return nc


# ---------------------------------------------------------------------------
# Host-side preparation
# ---------------------------------------------------------------------------

def prep_inputs(x, weight, bias, noise_const, noise_strength):
    SQ2 = np.sqrt(2.0)
    w = np.asarray(weight, np.float64)
    inv = 1.0 / np.sqrt((w ** 2).sum(axis=(1, 2, 3)) + 1e-8)
    w = w * inv[:, None, None, None]
    wf = w[:, :, ::-1, ::-1] * (SQ2 / 16.0)       # fold FIR norm + lrelu gain
    # wq[cg, ci, u*3+v, co]
    wq = np.ascontiguousarray(
        wf.transpose(1, 2, 3, 0).reshape(4, 128, 9, 256), dtype=np.float16)

    n2 = np.asarray(noise_const, np.float64) * float(noise_strength) * SQ2
    # noise part: [r, s*64 + m] = n2[r, 2m+s]; plus per-partition bias per ct
    nflat = n2.reshape(128, 64, 2).transpose(0, 2, 1).reshape(128, 128)
    b2 = (np.asarray(bias, np.float64) * SQ2).reshape(2, 128)
    nq = np.ascontiguousarray(
        nflat[None, None, :, :] + b2[:, :, None, None], dtype=np.float16)

    x = np.asarray(x, np.float32)
    maps = []
    for bi in range(x.shape[0]):
        xp = np.zeros((512, 67, 67), np.float16)
        xp[:, 1:65, 1:65] = x[bi]
        maps.append({
            "xin": xp.reshape(4, 128, 67, 67),
            "wq": wq,
            "nq": nq,
        })
    return maps


_NC_CACHE = None


def kernel(x, weight, bias, noise_const, noise_strength):
    global _NC_CACHE
    if _NC_CACHE is None:
        _NC_CACHE = build_conv_nc()
    in_maps = prep_inputs(x, weight, bias, noise_const, noise_strength)
    res = run_bass_kernel_spmd(_NC_CACHE, in_maps, core_ids=list(range(8)))
    return np.ascontiguousarray(
        np.stack([r["y"].reshape(256, 128, 128) for r in res.results]),
        dtype=np.float32)
